# revision 12
# baseline (speedup 1.0000x reference)
"""Trainium2 Bass kernel for nn_BasicAttentionBlock (8-core SPMD).

Math notes (validated against the reference in numpy first):

* The module is x + MHA1(LN(x)) + MHA2(LN(.), ctx) where ctx =
  relu(static @ Wse + bse) broadcast over time. Because every key/value row
  of attention-2 is identical per batch, softmax weights sum to 1 over equal
  value rows, so MHA2's output is exactly (ctx @ Wv2 + bv2) @ Wo2 broadcast
  over time — Q2/K2/scores2/softmax2 are skipped entirely.

* Attention-1 (16 heads, shared single V head, causal) is the real work.
  Sharding: batch b = core//4, and the 16 heads split 4-per-core within each
  batch group. The head-sum of attention vectors is reduce-scattered across
  the 4 cores of a batch group (token-sliced), and each core applies the
  out-projection + residual for its 512 tokens.

* LayerNorm is folded into the QKV projections: with W' = g⊙W,
  qT = rstd ⊙ (W'.T xT + (-colsum W')⊗m + (beta@W+b)⊗(1/rstd)).
  The two rank-1 corrections ride along as extra K=1 contraction rows; the
  per-token rstd scaling is one tensor_tensor multiply against a
  partition-broadcast rstd tile.

* Scores are computed transposed (S^T[key, q] on PE), exp on ScalarE with no
  max subtraction (|s| < ~4 by construction), causal handled by streaming
  only q >= key_block columns plus one 128x128 triangular mask multiply per
  diagonal block. p@v accumulates U^T[65, q] with a ones-column appended to
  V so row 64 collects the softmax denominators for free.
"""

import os
import sys

import numpy as np

if "/opt/trn_rl_repo" not in sys.path:
    sys.path.insert(0, "/opt/trn_rl_repo")

import ml_dtypes

BF = ml_dtypes.bfloat16

N_HEADS = 16
D_HEAD = 64
HID = 1024
T = 2048
BS = 2
LN_EPS = 1e-5
NCORES = 8
GROUP = 4  # cores per batch
HPC = N_HEADS // GROUP  # heads per core = 4
HCOLS = HPC * D_HEAD  # 256 projection cols per core
KT = HID // 128  # 8 k-tiles
NQT = T // 128  # 16 token tiles

_nc_cache = {}
last_results = None


def _build(flags):
    """Build the SPMD Bass program (same program for all 8 cores)."""
    has_cq, has_ck, has_cv, has_bse, has_bv2 = flags
    import concourse.bass as bass
    import concourse.tile as tile
    from concourse import bacc, mybir
    from concourse.masks import make_identity, make_upper_triangular

    f32 = mybir.dt.float32
    bf16 = mybir.dt.bfloat16
    AF = mybir.ActivationFunctionType
    ALU = mybir.AluOpType
    ts = bass.ts

    nc = bacc.Bacc("TRN2", target_bir_lowering=False)

    # ---- I/O ----
    xt_d = nc.dram_tensor("xt", [128, KT, T], bf16, kind="ExternalInput")
    xres_d = nc.dram_tensor("xres", [512, HID], f32, kind="ExternalInput")
    wq_d = nc.dram_tensor("wq", [128, KT, HCOLS], bf16, kind="ExternalInput")
    wk_d = nc.dram_tensor("wk", [128, KT, HCOLS], bf16, kind="ExternalInput")
    wv_d = nc.dram_tensor("wv", [128, KT, D_HEAD], bf16, kind="ExternalInput")
    sq_d = nc.dram_tensor("sq", [1, HCOLS], bf16, kind="ExternalInput")
    sk_d = nc.dram_tensor("sk", [1, HCOLS], bf16, kind="ExternalInput")
    sv_d = nc.dram_tensor("sv", [1, D_HEAD], bf16, kind="ExternalInput")
    cq_d = nc.dram_tensor("cq", [1, HCOLS], bf16, kind="ExternalInput") if has_cq else None
    ck_d = nc.dram_tensor("ck", [1, HCOLS], bf16, kind="ExternalInput") if has_ck else None
    cv_d = nc.dram_tensor("cv", [1, D_HEAD], bf16, kind="ExternalInput") if has_cv else None
    wo_d = nc.dram_tensor("wo", [D_HEAD, HID], bf16, kind="ExternalInput")
    sf_d = nc.dram_tensor("sf", [128, KT], bf16, kind="ExternalInput")
    wse_d = nc.dram_tensor("wse", [128, KT, HID], bf16, kind="ExternalInput")
    bse_d = nc.dram_tensor("bse", [1, HID], bf16, kind="ExternalInput") if has_bse else None
    wv2_d = nc.dram_tensor("wv2", [128, KT, D_HEAD], bf16, kind="ExternalInput")
    bv2_d = nc.dram_tensor("bv2", [1, D_HEAD], bf16, kind="ExternalInput") if has_bv2 else None
    wo2_d = nc.dram_tensor("wo2", [D_HEAD, HID], bf16, kind="ExternalInput")
    out_d = nc.dram_tensor("out", [512, HID], f32, kind="ExternalOutput")

    RG = [[0, 1, 2, 3], [4, 5, 6, 7]]

    with tile.TileContext(nc) as tc:
        with (
            tc.tile_pool(name="cpool", bufs=1) as cpool,
            tc.tile_pool(name="wpool", bufs=2) as wpool,
            tc.tile_pool(name="ps_a", bufs=3, space="PSUM") as ps_a,
            tc.tile_pool(name="ps_u", bufs=2, space="PSUM") as ps_u,
            tc.tile_pool(name="dpool", bufs=1, space="DRAM") as dpool,
        ):
            def ctile(shape, dt, name):
                return cpool.tile(shape, dt, name=name, tag=name)

            def wtile(shape, dt, name, tag, bufs):
                return wpool.tile(shape, dt, name=name, tag=tag, bufs=bufs)

            def patile(shape, name):
                return ps_a.tile(shape, f32, name=name, tag="a")

            def putile(shape, dt, name):
                return ps_u.tile(shape, dt, name=name, tag="u")

            # ---- constants / persistent SBUF ----
            xt_sb = ctile([128, KT, T], bf16, "xt_sb")
            nc.sync.dma_start(out=xt_sb, in_=xt_d[:, :, :])
            wq_sb = ctile([128, KT, HCOLS], bf16, "wq_sb")
            nc.sync.dma_start(out=wq_sb, in_=wq_d[:, :, :])
            wk_sb = ctile([128, KT, HCOLS], bf16, "wk_sb")
            nc.sync.dma_start(out=wk_sb, in_=wk_d[:, :, :])
            wv_sb = ctile([128, KT, D_HEAD], bf16, "wv_sb")
            nc.sync.dma_start(out=wv_sb, in_=wv_d[:, :, :])
            sq_sb = ctile([1, HCOLS], bf16, "sq_sb")
            nc.sync.dma_start(out=sq_sb, in_=sq_d[:, :])
            sk_sb = ctile([1, HCOLS], bf16, "sk_sb")
            nc.sync.dma_start(out=sk_sb, in_=sk_d[:, :])
            sv_sb = ctile([1, D_HEAD], bf16, "sv_sb")
            nc.sync.dma_start(out=sv_sb, in_=sv_d[:, :])
            cvec_sb = {}
            for nm, d in (("cq", cq_d), ("ck", ck_d), ("cv", cv_d)):
                if d is not None:
                    t_ = ctile([1, d.shape[1]], bf16, nm + "_sb")
                    nc.sync.dma_start(out=t_, in_=d[:, :])
                    cvec_sb[nm] = t_
            sf_sb = ctile([128, KT], bf16, "sf_sb")
            nc.sync.dma_start(out=sf_sb, in_=sf_d[:, :])
            wse_sb = ctile([128, KT, HID], bf16, "wse_sb")
            nc.sync.dma_start(out=wse_sb, in_=wse_d[:, :, :])
            if has_bse:
                bse_sb = ctile([1, HID], bf16, "bse_sb")
                nc.sync.dma_start(out=bse_sb, in_=bse_d[:, :])
            wv2_sb = ctile([128, KT, D_HEAD], bf16, "wv2_sb")
            nc.sync.dma_start(out=wv2_sb, in_=wv2_d[:, :, :])
            if has_bv2:
                bv2_sb = ctile([1, D_HEAD], bf16, "bv2_sb")
                nc.sync.dma_start(out=bv2_sb, in_=bv2_d[:, :])
            wo2_sb = ctile([D_HEAD, HID], bf16, "wo2_sb")
            nc.sync.dma_start(out=wo2_sb, in_=wo2_d[:, :])
            w_comb = ctile([D_HEAD + 1, HID], bf16, "w_comb")
            nc.sync.dma_start(out=w_comb[0:D_HEAD, :], in_=wo_d[:, :])
            xres_sb = ctile([128, 4, HID], f32, "xres_sb")
            for t_ in range(4):
                nc.sync.dma_start(out=xres_sb[:, t_, :], in_=xres_d[ts(t_, 128), :])

            ident = ctile([128, 128], bf16, "ident")
            make_identity(nc, ident)
            utri = ctile([128, 128], bf16, "utri")
            make_upper_triangular(nc, utri, val=1.0, diag=True)
            onesK = ctile([128, 1], bf16, "onesK")
            nc.vector.memset(onesK, 1.0 / HID)
            ones1 = ctile([1, 1], bf16, "ones1")
            nc.vector.memset(ones1, 1.0)
            eps_sb = ctile([1, 1], f32, "eps_sb")
            nc.vector.memset(eps_sb, LN_EPS)

            vrows = ctile([128, T], f32, "vrows")  # p0=rstd p32=m2 p64=var p96=lnv
            m_bf = ctile([1, T], bf16, "m_bf")
            invr_bf = ctile([1, T], bf16, "invr_bf")
            RSTD = ctile([128, T], f32, "RSTD")
            qT = [ctile([128, T], bf16, f"qT{i}") for i in range(2)]
            kT = [ctile([128, T], bf16, f"kT{i}") for i in range(2)]
            vT = ctile([D_HEAD, T], bf16, "vT")
            v_aug = ctile([128, NQT, D_HEAD + 1], bf16, "v_aug")
            nc.vector.memset(v_aug, 1.0)
            MT_acc = ctile([D_HEAD, T], f32, "MT_acc")
            MT_aug = ctile([D_HEAD + 1, 2, 256], bf16, "MT_aug")
            nc.vector.memset(MT_aug[D_HEAD : D_HEAD + 1, :, :], 1.0)
            ctxT_sb = ctile([128, KT], bf16, "ctxT_sb")

            # ---- static path: so2 = (relu(sf@Wse+bse) @ Wv2 + bv2) @ Wo2 ----
            ctx_ps = patile([1, HID], "ctx_ps")
            for n in range(2):
                c0 = n * 512
                for k in range(KT):
                    nc.tensor.matmul(
                        ctx_ps[0:1, c0 : c0 + 512],
                        sf_sb[:, k : k + 1],
                        wse_sb[:, k, c0 : c0 + 512],
                        start=(k == 0),
                        stop=(k == KT - 1 and not has_bse),
                    )
                if has_bse:
                    nc.tensor.matmul(
                        ctx_ps[0:1, c0 : c0 + 512],
                        ones1,
                        bse_sb[0:1, c0 : c0 + 512],
                        start=False,
                        stop=True,
                    )
            ctx_sb = ctile([1, HID], bf16, "ctx_sb")
            nc.scalar.activation(ctx_sb, ctx_ps, AF.Relu)
            ctx_bounce = dpool.tile([1, HID], bf16, name="ctx_bounce", tag="ctx_bounce")
            nc.gpsimd.dma_start(out=ctx_bounce[:, :], in_=ctx_sb[:, :])
            nc.gpsimd.dma_start(
                out=ctxT_sb[:, :], in_=ctx_bounce[0, :].rearrange("(k p) -> p k", p=128)
            )
            v2_ps = putile([1, D_HEAD], f32, "v2_ps")
            for k in range(KT):
                nc.tensor.matmul(
                    v2_ps,
                    ctxT_sb[:, k : k + 1],
                    wv2_sb[:, k, :],
                    start=(k == 0),
                    stop=(k == KT - 1 and not has_bv2),
                )
            if has_bv2:
                nc.tensor.matmul(v2_ps, ones1, bv2_sb[0:1, :], start=False, stop=True)
            v2_sb = ctile([1, D_HEAD], bf16, "v2_sb")
            nc.scalar.activation(v2_sb, v2_ps, AF.Copy)
            v2T_ps = putile([D_HEAD, 1], bf16, "v2T_ps")
            nc.tensor.transpose(v2T_ps, v2_sb[0:1, :], ident[0:1, 0:1])
            v2T_sb = ctile([D_HEAD, 1], bf16, "v2T_sb")
            nc.vector.tensor_copy(v2T_sb, v2T_ps)
            so2_ps = patile([1, HID], "so2_ps")
            for n in range(2):
                c0 = n * 512
                nc.tensor.matmul(
                    so2_ps[0:1, c0 : c0 + 512], v2T_sb, wo2_sb[:, c0 : c0 + 512],
                    start=True, stop=True,
                )
            nc.scalar.activation(w_comb[D_HEAD : D_HEAD + 1, :], so2_ps, AF.Copy)

            # ---- LN stats: m, E[x^2] via ones-matmuls over xT ----
            for half in range(2):
                hc0 = half * 1024
                m_ps = patile([1, 1024], f"m_ps{half}")
                ms_ps = patile([1, 1024], f"ms_ps{half}")
                for k in range(KT):
                    xsq = wtile([128, 1024], bf16, f"xsq{half}_{k}", "xsq", 2)
                    nc.vector.tensor_mul(
                        xsq, xt_sb[:, k, hc0 : hc0 + 1024], xt_sb[:, k, hc0 : hc0 + 1024]
                    )
                    for n in range(2):
                        c0 = n * 512
                        nc.tensor.matmul(
                            m_ps[0:1, c0 : c0 + 512],
                            onesK,
                            xt_sb[:, k, hc0 + c0 : hc0 + c0 + 512],
                            start=(k == 0),
                            stop=(k == KT - 1),
                        )
                        nc.tensor.matmul(
                            ms_ps[0:1, c0 : c0 + 512],
                            onesK,
                            xsq[:, c0 : c0 + 512],
                            start=(k == 0),
                            stop=(k == KT - 1),
                        )
                nc.scalar.activation(m_bf[0:1, hc0 : hc0 + 1024], m_ps, AF.Copy)
                nc.scalar.activation(vrows[32:33, hc0 : hc0 + 1024], m_ps, AF.Square)
                nc.vector.tensor_sub(
                    vrows[64:65, hc0 : hc0 + 1024], ms_ps, vrows[32:33, hc0 : hc0 + 1024]
                )
            nc.scalar.activation(vrows[96:97, :], vrows[64:65, :], AF.Ln, bias=eps_sb[0:1, 0:1])
            nc.scalar.activation(vrows[0:1, :], vrows[96:97, :], AF.Exp, scale=-0.5)
            nc.scalar.activation(invr_bf[0:1, :], vrows[96:97, :], AF.Exp, scale=0.5)
            nc.gpsimd.partition_broadcast(RSTD, vrows[0:1, :])

            # ---- projections qT/kT/vT (transposed, LN folded) ----
            projs = [
                ("q", wq_sb, sq_sb, cvec_sb.get("cq"), [qT[0], qT[1]], 128),
                ("k", wk_sb, sk_sb, cvec_sb.get("ck"), [kT[0], kT[1]], 128),
                ("v", wv_sb, sv_sb, cvec_sb.get("cv"), [vT], 64),
            ]
            for nm, w_sb, s_sb, c_sb, dests, P in projs:
                for mc, dest in enumerate(dests):
                    mcols = slice(mc * 128, mc * 128 + P)
                    for half in range(2):
                        hc0 = half * 1024
                        pp = patile([P, 1024], f"pp_{nm}{mc}{half}")
                        for n in range(2):
                            c0 = n * 512
                            for k in range(KT):
                                nc.tensor.matmul(
                                    pp[:, c0 : c0 + 512],
                                    w_sb[:, k, mcols],
                                    xt_sb[:, k, hc0 + c0 : hc0 + c0 + 512],
                                    start=(k == 0),
                                    stop=False,
                                )
                            nc.tensor.matmul(
                                pp[:, c0 : c0 + 512],
                                s_sb[0:1, mcols],
                                m_bf[0:1, hc0 + c0 : hc0 + c0 + 512],
                                start=False,
                                stop=(c_sb is None),
                            )
                            if c_sb is not None:
                                nc.tensor.matmul(
                                    pp[:, c0 : c0 + 512],
                                    c_sb[0:1, mcols],
                                    invr_bf[0:1, hc0 + c0 : hc0 + c0 + 512],
                                    start=False,
                                    stop=True,
                                )
                        nc.vector.tensor_mul(
                            dest[:P, hc0 : hc0 + 1024], pp, RSTD[:P, hc0 : hc0 + 1024]
                        )

            # ---- v_aug = [v | 1] in natural layout via PE transposes ----
            for t_ in range(NQT):
                vt_ps = putile([128, D_HEAD], bf16, f"vt_ps{t_}")
                nc.tensor.transpose(vt_ps, vT[0:D_HEAD, ts(t_, 128)], ident[0:D_HEAD, 0:D_HEAD])
                nc.vector.tensor_copy(v_aug[:, t_, 0:D_HEAD], vt_ps)

            # ---- attention: per qc half / head / key block (pipelined) ----
            for qc in range(2):
                qbase = qc * 1024
                for hp in range(2):
                    for hb in range(2):
                        rb = hb * 64
                        U = [putile([D_HEAD + 1, 512], f32, f"U{qc}{hp}{hb}{s}") for s in range(2)]
                        jmax = 8 if qc == 0 else 16
                        for j in range(jmax):
                            qs = max(qbase, j * 128)
                            N = qbase + 1024 - qs
                            S = patile([128, N], f"S{qc}{hp}{j}{hb}")
                            for n0 in range(0, N, 512):
                                n1 = min(n0 + 512, N)
                                nc.tensor.matmul(
                                    S[:, n0:n1],
                                    kT[hp][rb : rb + 64, ts(j, 128)],
                                    qT[hp][rb : rb + 64, qs + n0 : qs + n1],
                                    start=True,
                                    stop=True,
                                )
                            E = wtile([128, N], bf16, f"E{qc}{hp}{j}{hb}", "E", 3)
                            nc.scalar.activation(E, S, AF.Exp)
                            if j >= qc * 8:
                                nc.vector.tensor_mul(E[:, 0:128], E[:, 0:128], utri)
                            for s in range(2):
                                sub0 = qbase + 512 * s
                                sub1 = sub0 + 512
                                lo = max(qs, sub0)
                                if lo >= sub1:
                                    continue
                                e0 = lo - qs
                                u0 = lo - sub0
                                n = sub1 - lo
                                j_last = min(jmax - 1, (sub1 - 1) // 128)
                                nc.tensor.matmul(
                                    U[s][:, u0 : u0 + n],
                                    v_aug[:, j, :],
                                    E[:, e0 : e0 + n],
                                    start=(j == 0),
                                    stop=(j == j_last),
                                    skip_group_check=True,
                                )
                        first_head = hp == 0 and hb == 0
                        for s in range(2):
                            gs = slice(qbase + 512 * s, qbase + 512 * s + 512)
                            dn = wtile([1, 512], f32, f"dn{qc}{hp}{hb}{s}", "dn", 2)
                            with nc.allow_low_precision("softmax denom"):
                                nc.vector.reciprocal(dn, U[s][D_HEAD : D_HEAD + 1, :])
                            bc = wtile([64, 512], f32, f"bc{qc}{hp}{hb}{s}", "bc", 2)
                            nc.gpsimd.partition_broadcast(bc, dn[0:1, :])
                            if first_head:
                                nc.vector.tensor_mul(MT_acc[:, gs], U[s][0:D_HEAD, :], bc)
                            else:
                                tmp = wtile([64, 512], f32, f"tmp{qc}{hp}{hb}{s}", "tmp", 2)
                                nc.vector.tensor_mul(tmp, U[s][0:D_HEAD, :], bc)
                                nc.vector.tensor_add(MT_acc[:, gs], MT_acc[:, gs], tmp)

                # ---- reduce-scatter this half's head-sum, out-proj our tokens ----
                cc_in = dpool.tile([4, D_HEAD, 256], f32, name=f"cc_in{qc}", tag=f"cc_in{qc}")
                for blk in range(4):
                    nc.gpsimd.dma_start(
                        out=cc_in[blk], in_=MT_acc[:, qbase + 256 * blk : qbase + 256 * blk + 256]
                    )
                cc_out = dpool.tile([D_HEAD, 256], f32, name=f"cc_out{qc}", tag=f"cc_out{qc}")
                nc.gpsimd.collective_compute(
                    "ReduceScatter",
                    ALU.add,
                    replica_groups=RG,
                    ins=[cc_in.opt()],
                    outs=[cc_out.opt()],
                )
                mt_st = wtile([D_HEAD, 256], f32, f"mt_st{qc}", "mt_st", 2)
                nc.sync.dma_start(out=mt_st, in_=cc_out[:, :])
                nc.vector.tensor_copy(MT_aug[0:D_HEAD, qc, :], mt_st)
                for tt in range(2):
                    o_ps = patile([128, HID], f"o_ps{qc}{tt}")
                    for n in range(2):
                        c0 = n * 512
                        nc.tensor.matmul(
                            o_ps[:, c0 : c0 + 512],
                            MT_aug[:, qc, ts(tt, 128)],
                            w_comb[:, c0 : c0 + 512],
                            start=True,
                            stop=True,
                        )
                    out_sb = wtile([128, HID], f32, f"out_sb{qc}{tt}", "out_sb", 2)
                    nc.vector.tensor_add(out_sb, o_ps, xres_sb[:, qc * 2 + tt, :])
                    nc.sync.dma_start(out=out_d[ts(qc * 2 + tt, 128), :], in_=out_sb)

    nc.compile()
    return nc


def _get_nc(flags):
    if flags not in _nc_cache:
        _nc_cache[flags] = _build(flags)
    return _nc_cache[flags]


def _prep_core_inputs(c, arrs, flags):
    has_cq, has_ck, has_cv, has_bse, has_bv2 = flags
    b, p = divmod(c, GROUP)
    x = arrs["x"][b]  # [T, HID] f32
    g1 = arrs["g1"]
    hs = slice(p * HCOLS, (p + 1) * HCOLS)

    def kmajor(w):  # [HID, C] -> [128, KT, C]
        return np.ascontiguousarray(
            w.reshape(KT, 128, w.shape[1]).transpose(1, 0, 2)
        ).astype(BF)

    wq_full = (g1[:, None] * arrs["Wq1"]) / 8.0
    wk_full = g1[:, None] * arrs["Wk1"]
    wv_full = g1[:, None] * arrs["Wv1"]
    wq = wq_full[:, hs]
    wk = wk_full[:, hs]
    cq_full = (arrs["beta1"] @ arrs["Wq1"] + arrs["bq1"]) / 8.0
    ck_full = arrs["beta1"] @ arrs["Wk1"] + arrs["bk1"]
    cv_full = arrs["beta1"] @ arrs["Wv1"] + arrs["bv1"]

    rows = np.r_[256 * p : 256 * p + 256, 1024 + 256 * p : 1024 + 256 * p + 256]
    d = {
        "xt": np.ascontiguousarray(
            x.T.reshape(KT, 128, T).transpose(1, 0, 2)
        ).astype(BF),
        "xres": np.ascontiguousarray(x[rows]).astype(np.float32),
        "wq": kmajor(wq),
        "wk": kmajor(wk),
        "wv": kmajor(wv_full),
        "sq": (-wq.astype(np.float64).sum(0))[None].astype(BF),
        "sk": (-wk.astype(np.float64).sum(0))[None].astype(BF),
        "sv": (-wv_full.astype(np.float64).sum(0))[None].astype(BF),
        "wo": (arrs["Wo1"] / float(N_HEADS)).astype(BF),
        "sf": np.ascontiguousarray(arrs["static_features"][b].reshape(KT, 128).T).astype(BF),
        "wse": kmajor(arrs["Wse"]),
        "wv2": kmajor(arrs["Wv2"]),
        "wo2": arrs["Wo2"].astype(BF),
    }
    if has_cq:
        d["cq"] = cq_full[hs][None].astype(BF)
    if has_ck:
        d["ck"] = ck_full[hs][None].astype(BF)
    if has_cv:
        d["cv"] = cv_full[None].astype(BF)
    if has_bse:
        d["bse"] = arrs["bse"][None].astype(BF)
    if has_bv2:
        d["bv2"] = arrs["bv2"][None].astype(BF)
    return d


def kernel(**inputs):
    global last_results
    arrs = {k: np.asarray(v, np.float32) for k, v in inputs.items()}

    cq_full = (arrs["beta1"] @ arrs["Wq1"] + arrs["bq1"]) / 8.0
    ck_full = arrs["beta1"] @ arrs["Wk1"] + arrs["bk1"]
    cv_full = arrs["beta1"] @ arrs["Wv1"] + arrs["bv1"]
    flags = (
        bool(np.any(cq_full != 0)),
        bool(np.any(ck_full != 0)),
        bool(np.any(cv_full != 0)),
        bool(np.any(arrs["bse"] != 0)),
        bool(np.any(arrs["bv2"] != 0)),
    )
    nc = _get_nc(flags)

    in_maps = [_prep_core_inputs(c, arrs, flags) for c in range(NCORES)]

    from concourse.bass_utils import run_bass_kernel_spmd

    kw = {}
    prof_dir = os.environ.get("BASS_PROF_DIR")
    if prof_dir:
        os.makedirs(prof_dir, exist_ok=True)
        kw["tmpdir"] = prof_dir
    res = run_bass_kernel_spmd(nc, in_maps, list(range(NCORES)), **kw)
    last_results = res

    out = np.empty((BS, T, HID), np.float32)
    for c in range(NCORES):
        b, p = divmod(c, GROUP)
        o = np.asarray(res.results[c]["out"], np.float32)
        out[b, 256 * p : 256 * p + 256] = o[0:256]
        out[b, 1024 + 256 * p : 1024 + 256 * p + 256] = o[256:512]
    return out


# revision 14
# speedup vs baseline: 1.4259x; 1.4259x over previous
"""Trainium2 Bass kernel for nn_BasicAttentionBlock (8-core SPMD).

Math notes (validated against the reference in numpy first):

* The module is x + MHA1(LN(x)) + MHA2(LN(.), ctx) where ctx =
  relu(static @ Wse + bse) broadcast over time. Because every key/value row
  of attention-2 is identical per batch, softmax weights sum to 1 over equal
  value rows, so MHA2's output is exactly (ctx @ Wv2 + bv2) @ Wo2 broadcast
  over time — Q2/K2/scores2/softmax2 are skipped entirely.

* Attention-1 (16 heads, shared single V head, causal) is the real work.
  Sharding: batch b = core//4, and the 16 heads split 4-per-core within each
  batch group. The head-sum of attention vectors is reduce-scattered across
  the 4 cores of a batch group (token-sliced), and each core applies the
  out-projection + residual for its 512 tokens.

* LayerNorm is folded into the QKV projections: with W' = g⊙W,
  qT = rstd ⊙ (W'.T xT + (-colsum W')⊗m + (beta@W+b)⊗(1/rstd)).
  The two rank-1 corrections ride along as extra K=1 contraction rows; the
  per-token rstd scaling is one tensor_tensor multiply against a
  partition-broadcast rstd tile.

* Scores are computed transposed (S^T[key, q] on PE), exp on ScalarE with no
  max subtraction (|s| < ~4 by construction), causal handled by streaming
  only q >= key_block columns plus one 128x128 triangular mask multiply per
  diagonal block. p@v accumulates U^T[65, q] with a ones-column appended to
  V so row 64 collects the softmax denominators for free.
"""

import os
import sys

import numpy as np

if "/opt/trn_rl_repo" not in sys.path:
    sys.path.insert(0, "/opt/trn_rl_repo")

import ml_dtypes

BF = ml_dtypes.bfloat16

N_HEADS = 16
D_HEAD = 64
HID = 1024
T = 2048
BS = 2
LN_EPS = 1e-5
NCORES = 8
GROUP = 4  # cores per batch
HPC = N_HEADS // GROUP  # heads per core = 4
HCOLS = HPC * D_HEAD  # 256 projection cols per core
KT = HID // 128  # 8 k-tiles
NQT = T // 128  # 16 token tiles

_nc_cache = {}
last_results = None


def _build(flags):
    """Build the SPMD Bass program (same program for all 8 cores)."""
    has_cq, has_ck, has_cv, has_bse, has_bv2 = flags
    import concourse.bass as bass
    import concourse.tile as tile
    from concourse import bacc, mybir
    from concourse.masks import make_identity, make_upper_triangular

    f32 = mybir.dt.float32
    bf16 = mybir.dt.bfloat16
    AF = mybir.ActivationFunctionType
    ALU = mybir.AluOpType
    ts = bass.ts

    nc = bacc.Bacc("TRN2", target_bir_lowering=False)

    # ---- I/O ----
    xt_d = nc.dram_tensor("xt", [128, KT, T], bf16, kind="ExternalInput")
    xres_d = nc.dram_tensor("xres", [512, HID], f32, kind="ExternalInput")
    wq_d = nc.dram_tensor("wq", [128, KT, HCOLS], bf16, kind="ExternalInput")
    wk_d = nc.dram_tensor("wk", [128, KT, HCOLS], bf16, kind="ExternalInput")
    wv_d = nc.dram_tensor("wv", [128, KT, D_HEAD], bf16, kind="ExternalInput")
    sq_d = nc.dram_tensor("sq", [1, HCOLS], bf16, kind="ExternalInput")
    sk_d = nc.dram_tensor("sk", [1, HCOLS], bf16, kind="ExternalInput")
    sv_d = nc.dram_tensor("sv", [1, D_HEAD], bf16, kind="ExternalInput")
    cq_d = nc.dram_tensor("cq", [1, HCOLS], bf16, kind="ExternalInput") if has_cq else None
    ck_d = nc.dram_tensor("ck", [1, HCOLS], bf16, kind="ExternalInput") if has_ck else None
    cv_d = nc.dram_tensor("cv", [1, D_HEAD], bf16, kind="ExternalInput") if has_cv else None
    wo_d = nc.dram_tensor("wo", [D_HEAD, HID], bf16, kind="ExternalInput")
    sf_d = nc.dram_tensor("sf", [128, KT], bf16, kind="ExternalInput")
    wse_d = nc.dram_tensor("wse", [128, KT, HID], bf16, kind="ExternalInput")
    bse_d = nc.dram_tensor("bse", [1, HID], bf16, kind="ExternalInput") if has_bse else None
    wv2_d = nc.dram_tensor("wv2", [128, KT, D_HEAD], bf16, kind="ExternalInput")
    bv2_d = nc.dram_tensor("bv2", [1, D_HEAD], bf16, kind="ExternalInput") if has_bv2 else None
    wo2_d = nc.dram_tensor("wo2", [D_HEAD, HID], bf16, kind="ExternalInput")
    out_d = nc.dram_tensor("out", [512, HID], f32, kind="ExternalOutput")

    RG = [[0, 1, 2, 3], [4, 5, 6, 7]]

    with tile.TileContext(nc) as tc:
        with (
            tc.tile_pool(name="cpool", bufs=1) as cpool,
            tc.tile_pool(name="wpool", bufs=2) as wpool,
            tc.tile_pool(name="dpool", bufs=1, space="DRAM") as dpool,
        ):
            from contextlib import ExitStack

            _phase_a = ExitStack()
            ps_a = _phase_a.enter_context(tc.tile_pool(name="ps_a", bufs=3, space="PSUM"))
            ps_u = None  # opened for the attention phase after phase A closes
            def ctile(shape, dt, name):
                return cpool.tile(shape, dt, name=name, tag=name)

            def wtile(shape, dt, name, tag, bufs):
                return wpool.tile(shape, dt, name=name, tag=tag, bufs=bufs)

            def patile(shape, name):
                return ps_a.tile(shape, f32, name=name, tag="a")

            def putile(shape, dt, name):
                return ps_u.tile(shape, dt, name=name, tag="u")

            def pstile(shape, dt, name):
                return ps_s.tile(shape, dt, name=name, tag="s")

            # ---- constants / persistent SBUF ----
            xt_sb = ctile([128, KT, T], bf16, "xt_sb")
            nc.sync.dma_start(out=xt_sb, in_=xt_d[:, :, :])
            wq_sb = ctile([128, KT, HCOLS], bf16, "wq_sb")
            nc.sync.dma_start(out=wq_sb, in_=wq_d[:, :, :])
            wk_sb = ctile([128, KT, HCOLS], bf16, "wk_sb")
            nc.sync.dma_start(out=wk_sb, in_=wk_d[:, :, :])
            wv_sb = ctile([128, KT, D_HEAD], bf16, "wv_sb")
            nc.sync.dma_start(out=wv_sb, in_=wv_d[:, :, :])
            sq_sb = ctile([1, HCOLS], bf16, "sq_sb")
            nc.sync.dma_start(out=sq_sb, in_=sq_d[:, :])
            sk_sb = ctile([1, HCOLS], bf16, "sk_sb")
            nc.sync.dma_start(out=sk_sb, in_=sk_d[:, :])
            sv_sb = ctile([1, D_HEAD], bf16, "sv_sb")
            nc.sync.dma_start(out=sv_sb, in_=sv_d[:, :])
            cvec_sb = {}
            for nm, d in (("cq", cq_d), ("ck", ck_d), ("cv", cv_d)):
                if d is not None:
                    t_ = ctile([1, d.shape[1]], bf16, nm + "_sb")
                    nc.sync.dma_start(out=t_, in_=d[:, :])
                    cvec_sb[nm] = t_
            sf_sb = ctile([128, KT], bf16, "sf_sb")
            nc.sync.dma_start(out=sf_sb, in_=sf_d[:, :])
            wse_sb = ctile([128, KT, HID], bf16, "wse_sb")
            nc.sync.dma_start(out=wse_sb, in_=wse_d[:, :, :])
            if has_bse:
                bse_sb = ctile([1, HID], bf16, "bse_sb")
                nc.sync.dma_start(out=bse_sb, in_=bse_d[:, :])
            wv2_sb = ctile([128, KT, D_HEAD], bf16, "wv2_sb")
            nc.sync.dma_start(out=wv2_sb, in_=wv2_d[:, :, :])
            if has_bv2:
                bv2_sb = ctile([1, D_HEAD], bf16, "bv2_sb")
                nc.sync.dma_start(out=bv2_sb, in_=bv2_d[:, :])
            wo2_sb = ctile([D_HEAD, HID], bf16, "wo2_sb")
            nc.sync.dma_start(out=wo2_sb, in_=wo2_d[:, :])
            w_comb = ctile([D_HEAD + 1, HID], bf16, "w_comb")
            nc.sync.dma_start(out=w_comb[0:D_HEAD, :], in_=wo_d[:, :])
            xres_sb = ctile([128, 4, HID], f32, "xres_sb")
            for t_ in range(4):
                nc.sync.dma_start(out=xres_sb[:, t_, :], in_=xres_d[ts(t_, 128), :])

            ident = ctile([128, 128], bf16, "ident")
            make_identity(nc, ident)
            utri = ctile([128, 128], bf16, "utri")
            make_upper_triangular(nc, utri, val=1.0, diag=True)
            onesK = ctile([128, 1], bf16, "onesK")
            nc.vector.memset(onesK, 1.0 / HID)
            ones1 = ctile([1, 1], bf16, "ones1")
            nc.vector.memset(ones1, 1.0)
            eps_sb = ctile([1, 1], f32, "eps_sb")
            nc.vector.memset(eps_sb, LN_EPS)

            vrows = ctile([128, T], f32, "vrows")  # p0=rstd p32=m2 p64=var p96=lnv
            m_bf = ctile([1, T], bf16, "m_bf")
            invr_bf = ctile([1, T], bf16, "invr_bf")
            RSTD = ctile([128, T], f32, "RSTD")
            qT = [ctile([128, T], bf16, f"qT{i}") for i in range(2)]
            kT = [ctile([128, T], bf16, f"kT{i}") for i in range(2)]
            vT = ctile([D_HEAD, T], bf16, "vT")
            v_aug = ctile([128, NQT, D_HEAD + 1], bf16, "v_aug")
            nc.vector.memset(v_aug, 1.0)
            MT_acc = ctile([D_HEAD, T], f32, "MT_acc")
            MT_aug = ctile([D_HEAD + 1, 2, 256], bf16, "MT_aug")
            nc.vector.memset(MT_aug[D_HEAD : D_HEAD + 1, :, :], 1.0)
            ctxT_sb = ctile([128, KT], bf16, "ctxT_sb")

            # ---- static path: so2 = (relu(sf@Wse+bse) @ Wv2 + bv2) @ Wo2 ----
            ctx_ps = patile([1, HID], "ctx_ps")
            for n in range(2):
                c0 = n * 512
                for k in range(KT):
                    nc.tensor.matmul(
                        ctx_ps[0:1, c0 : c0 + 512],
                        sf_sb[:, k : k + 1],
                        wse_sb[:, k, c0 : c0 + 512],
                        start=(k == 0),
                        stop=(k == KT - 1 and not has_bse),
                    )
                if has_bse:
                    nc.tensor.matmul(
                        ctx_ps[0:1, c0 : c0 + 512],
                        ones1,
                        bse_sb[0:1, c0 : c0 + 512],
                        start=False,
                        stop=True,
                    )
            ctx_sb = ctile([1, HID], bf16, "ctx_sb")
            nc.scalar.activation(ctx_sb, ctx_ps, AF.Relu)
            ctx_bounce = dpool.tile([1, HID], bf16, name="ctx_bounce", tag="ctx_bounce")
            nc.gpsimd.dma_start(out=ctx_bounce[:, :], in_=ctx_sb[:, :])
            nc.gpsimd.dma_start(
                out=ctxT_sb[:, :], in_=ctx_bounce[0, :].rearrange("(k p) -> p k", p=128)
            )
            v2_ps = ps_a.tile([1, D_HEAD], f32, name="v2_ps", tag="a")
            for k in range(KT):
                nc.tensor.matmul(
                    v2_ps,
                    ctxT_sb[:, k : k + 1],
                    wv2_sb[:, k, :],
                    start=(k == 0),
                    stop=(k == KT - 1 and not has_bv2),
                )
            if has_bv2:
                nc.tensor.matmul(v2_ps, ones1, bv2_sb[0:1, :], start=False, stop=True)
            v2_sb = ctile([1, D_HEAD], bf16, "v2_sb")
            nc.scalar.activation(v2_sb, v2_ps, AF.Copy)
            v2T_ps = ps_a.tile([D_HEAD, 1], bf16, name="v2T_ps", tag="a")
            nc.tensor.transpose(v2T_ps, v2_sb[0:1, :], ident[0:1, 0:1])
            v2T_sb = ctile([D_HEAD, 1], bf16, "v2T_sb")
            nc.vector.tensor_copy(v2T_sb, v2T_ps)
            so2_ps = patile([1, HID], "so2_ps")
            for n in range(2):
                c0 = n * 512
                nc.tensor.matmul(
                    so2_ps[0:1, c0 : c0 + 512], v2T_sb, wo2_sb[:, c0 : c0 + 512],
                    start=True, stop=True,
                )
            nc.scalar.activation(w_comb[D_HEAD : D_HEAD + 1, :], so2_ps, AF.Copy)

            # ---- LN stats: m, E[x^2] via ones-matmuls over xT ----
            for half in range(2):
                hc0 = half * 1024
                m_ps = patile([1, 1024], f"m_ps{half}")
                ms_ps = patile([1, 1024], f"ms_ps{half}")
                for k in range(KT):
                    xsq = wtile([128, 1024], bf16, f"xsq{half}_{k}", "xsq", 2)
                    nc.vector.tensor_mul(
                        xsq, xt_sb[:, k, hc0 : hc0 + 1024], xt_sb[:, k, hc0 : hc0 + 1024]
                    )
                    for n in range(2):
                        c0 = n * 512
                        nc.tensor.matmul(
                            m_ps[0:1, c0 : c0 + 512],
                            onesK,
                            xt_sb[:, k, hc0 + c0 : hc0 + c0 + 512],
                            start=(k == 0),
                            stop=(k == KT - 1),
                        )
                        nc.tensor.matmul(
                            ms_ps[0:1, c0 : c0 + 512],
                            onesK,
                            xsq[:, c0 : c0 + 512],
                            start=(k == 0),
                            stop=(k == KT - 1),
                        )
                nc.scalar.activation(m_bf[0:1, hc0 : hc0 + 1024], m_ps, AF.Copy)
                nc.scalar.activation(vrows[32:33, hc0 : hc0 + 1024], m_ps, AF.Square)
                nc.vector.tensor_sub(
                    vrows[64:65, hc0 : hc0 + 1024], ms_ps, vrows[32:33, hc0 : hc0 + 1024]
                )
            nc.scalar.activation(vrows[96:97, :], vrows[64:65, :], AF.Ln, bias=eps_sb[0:1, 0:1])
            nc.scalar.activation(vrows[0:1, :], vrows[96:97, :], AF.Exp, scale=-0.5)
            nc.scalar.activation(invr_bf[0:1, :], vrows[96:97, :], AF.Exp, scale=0.5)
            nc.gpsimd.partition_broadcast(RSTD, vrows[0:1, :])

            # ---- projections qT/kT/vT (transposed, LN folded) ----
            projs = [
                ("q", wq_sb, sq_sb, cvec_sb.get("cq"), [qT[0], qT[1]], 128),
                ("k", wk_sb, sk_sb, cvec_sb.get("ck"), [kT[0], kT[1]], 128),
                ("v", wv_sb, sv_sb, cvec_sb.get("cv"), [vT], 64),
            ]
            for nm, w_sb, s_sb, c_sb, dests, P in projs:
                for mc, dest in enumerate(dests):
                    mcols = slice(mc * 128, mc * 128 + P)
                    for half in range(2):
                        hc0 = half * 1024
                        pp = patile([P, 1024], f"pp_{nm}{mc}{half}")
                        for n in range(2):
                            c0 = n * 512
                            for k in range(KT):
                                nc.tensor.matmul(
                                    pp[:, c0 : c0 + 512],
                                    w_sb[:, k, mcols],
                                    xt_sb[:, k, hc0 + c0 : hc0 + c0 + 512],
                                    start=(k == 0),
                                    stop=False,
                                )
                            nc.tensor.matmul(
                                pp[:, c0 : c0 + 512],
                                s_sb[0:1, mcols],
                                m_bf[0:1, hc0 + c0 : hc0 + c0 + 512],
                                start=False,
                                stop=(c_sb is None),
                            )
                            if c_sb is not None:
                                nc.tensor.matmul(
                                    pp[:, c0 : c0 + 512],
                                    c_sb[0:1, mcols],
                                    invr_bf[0:1, hc0 + c0 : hc0 + c0 + 512],
                                    start=False,
                                    stop=True,
                                )
                        nc.vector.tensor_mul(
                            dest[:P, hc0 : hc0 + 1024], pp, RSTD[:P, hc0 : hc0 + 1024]
                        )

            # ---- v_aug = [v | 1] in natural layout via PE transposes ----
            for t_ in range(NQT):
                vt_ps = ps_a.tile([128, D_HEAD], bf16, name=f"vt_ps{t_}", tag="a")
                nc.tensor.transpose(vt_ps, vT[0:D_HEAD, ts(t_, 128)], ident[0:D_HEAD, 0:D_HEAD])
                nc.vector.tensor_copy(v_aug[:, t_, 0:D_HEAD], vt_ps)

            # ---- attention (phase B): 512-col q-chunks, head-pair row-packed ----
            _phase_a.close()
            _phase_b = ExitStack()
            ps_s = _phase_b.enter_context(tc.tile_pool(name="ps_s", bufs=2, space="PSUM"))
            ps_u = _phase_b.enter_context(tc.tile_pool(name="ps_u", bufs=4, space="PSUM"))

            def do_outproj(qc):
                cc_in = dpool.tile([4, D_HEAD, 256], f32, name=f"cc_in{qc}", tag=f"cc_in{qc}")
                qb = qc * 1024
                for blk in range(4):
                    nc.gpsimd.dma_start(
                        out=cc_in[blk], in_=MT_acc[:, qb + 256 * blk : qb + 256 * blk + 256]
                    )
                cc_out = dpool.tile([D_HEAD, 256], f32, name=f"cc_out{qc}", tag=f"cc_out{qc}")
                nc.gpsimd.collective_compute(
                    "ReduceScatter",
                    ALU.add,
                    replica_groups=RG,
                    ins=[cc_in.opt()],
                    outs=[cc_out.opt()],
                )
                mt_st = wtile([D_HEAD, 256], f32, f"mt_st{qc}", "mt_st", 2)
                nc.sync.dma_start(out=mt_st, in_=cc_out[:, :])
                nc.vector.tensor_copy(MT_aug[0:D_HEAD, qc, :], mt_st)
                for tt in range(2):
                    out_sb = wtile([128, HID], f32, f"out_sb{qc}{tt}", "out_sb", 2)
                    for n in range(2):
                        c0 = n * 512
                        o_ps = putile([128, 512], f32, f"o_ps{qc}{tt}{n}")
                        nc.tensor.matmul(
                            o_ps,
                            MT_aug[:, qc, ts(tt, 128)],
                            w_comb[:, c0 : c0 + 512],
                            start=True,
                            stop=True,
                        )
                        nc.vector.tensor_add(
                            out_sb[:, c0 : c0 + 512], o_ps, xres_sb[:, qc * 2 + tt, c0 : c0 + 512]
                        )
                    nc.sync.dma_start(out=out_d[ts(qc * 2 + tt, 128), :], in_=out_sb)

            for c in range(4):
                cbase = 512 * c
                for hp in range(2):
                    U = [
                        [putile([D_HEAD + 1, 512], f32, f"U{c}{hp}{hb}{s}") for s in range(1)]
                        for hb in range(2)
                    ]
                    jmax = 4 * c + 4
                    for j in range(jmax):
                        qs = max(cbase, j * 128)
                        N = cbase + 512 - qs
                        Sp = pstile([128, 1024], f32, f"S{c}{hp}{j}")
                        for hb in range(2):
                            rb = hb * 64
                            nc.tensor.matmul(
                                Sp[:, hb * 512 : hb * 512 + N],
                                kT[hp][rb : rb + 64, ts(j, 128)],
                                qT[hp][rb : rb + 64, qs : qs + N],
                                start=True,
                                stop=True,
                            )
                        E = wtile([128, 1024], bf16, f"E{c}{hp}{j}", "E", 3)
                        if N == 512:
                            nc.scalar.activation(E, Sp, AF.Exp)
                        else:
                            nc.scalar.activation(E[:, 0:N], Sp[:, 0:N], AF.Exp)
                            nc.scalar.activation(
                                E[:, 512 : 512 + N], Sp[:, 512 : 512 + N], AF.Exp
                            )
                        if j >= 4 * c:
                            nc.vector.tensor_mul(E[:, 0:128], E[:, 0:128], utri)
                            nc.vector.tensor_mul(E[:, 512:640], E[:, 512:640], utri)
                        u0 = qs - cbase
                        j_last = jmax - 1
                        for hb in range(2):
                            nc.tensor.matmul(
                                U[hb][0][:, u0 : u0 + N],
                                v_aug[:, j, :],
                                E[:, hb * 512 : hb * 512 + N],
                                start=(j == 0),
                                stop=(j == j_last),
                                skip_group_check=True,
                            )
                    for hb in range(2):
                        first_head = hp == 0 and hb == 0
                        gs = slice(cbase, cbase + 512)
                        dnr = wtile([1, 512], f32, f"dnr{c}{hp}{hb}", "dnr", 2)
                        nc.vector.tensor_copy(dnr, U[hb][0][D_HEAD : D_HEAD + 1, :])
                        dn = wtile([1, 512], f32, f"dn{c}{hp}{hb}", "dn", 2)
                        nc.vector.reciprocal_approx_fast(dn, dnr)
                        bc = wtile([64, 512], f32, f"bc{c}{hp}{hb}", "bc", 2)
                        nc.gpsimd.partition_broadcast(bc, dn[0:1, :])
                        if first_head:
                            nc.vector.tensor_mul(MT_acc[:, gs], U[hb][0][0:D_HEAD, :], bc)
                        else:
                            tmp = wtile([64, 512], f32, f"tmp{c}{hp}{hb}", "tmp", 2)
                            nc.vector.tensor_mul(tmp, U[hb][0][0:D_HEAD, :], bc)
                            nc.vector.tensor_add(MT_acc[:, gs], MT_acc[:, gs], tmp)
                if c == 1:
                    do_outproj(0)
            do_outproj(1)
            _phase_b.close()

    nc.compile()
    return nc


def _get_nc(flags):
    if flags not in _nc_cache:
        _nc_cache[flags] = _build(flags)
    return _nc_cache[flags]


def _prep_core_inputs(c, arrs, flags):
    has_cq, has_ck, has_cv, has_bse, has_bv2 = flags
    b, p = divmod(c, GROUP)
    x = arrs["x"][b]  # [T, HID] f32
    g1 = arrs["g1"]
    hs = slice(p * HCOLS, (p + 1) * HCOLS)

    def kmajor(w):  # [HID, C] -> [128, KT, C]
        return np.ascontiguousarray(
            w.reshape(KT, 128, w.shape[1]).transpose(1, 0, 2)
        ).astype(BF)

    wq_full = (g1[:, None] * arrs["Wq1"]) / 8.0
    wk_full = g1[:, None] * arrs["Wk1"]
    wv_full = g1[:, None] * arrs["Wv1"]
    wq = wq_full[:, hs]
    wk = wk_full[:, hs]
    cq_full = (arrs["beta1"] @ arrs["Wq1"] + arrs["bq1"]) / 8.0
    ck_full = arrs["beta1"] @ arrs["Wk1"] + arrs["bk1"]
    cv_full = arrs["beta1"] @ arrs["Wv1"] + arrs["bv1"]

    rows = np.r_[256 * p : 256 * p + 256, 1024 + 256 * p : 1024 + 256 * p + 256]
    d = {
        "xt": np.ascontiguousarray(
            x.T.reshape(KT, 128, T).transpose(1, 0, 2)
        ).astype(BF),
        "xres": np.ascontiguousarray(x[rows]).astype(np.float32),
        "wq": kmajor(wq),
        "wk": kmajor(wk),
        "wv": kmajor(wv_full),
        "sq": (-wq.astype(np.float64).sum(0))[None].astype(BF),
        "sk": (-wk.astype(np.float64).sum(0))[None].astype(BF),
        "sv": (-wv_full.astype(np.float64).sum(0))[None].astype(BF),
        "wo": (arrs["Wo1"] / float(N_HEADS)).astype(BF),
        "sf": np.ascontiguousarray(arrs["static_features"][b].reshape(KT, 128).T).astype(BF),
        "wse": kmajor(arrs["Wse"]),
        "wv2": kmajor(arrs["Wv2"]),
        "wo2": arrs["Wo2"].astype(BF),
    }
    if has_cq:
        d["cq"] = cq_full[hs][None].astype(BF)
    if has_ck:
        d["ck"] = ck_full[hs][None].astype(BF)
    if has_cv:
        d["cv"] = cv_full[None].astype(BF)
    if has_bse:
        d["bse"] = arrs["bse"][None].astype(BF)
    if has_bv2:
        d["bv2"] = arrs["bv2"][None].astype(BF)
    return d


def kernel(**inputs):
    global last_results
    arrs = {k: np.asarray(v, np.float32) for k, v in inputs.items()}

    cq_full = (arrs["beta1"] @ arrs["Wq1"] + arrs["bq1"]) / 8.0
    ck_full = arrs["beta1"] @ arrs["Wk1"] + arrs["bk1"]
    cv_full = arrs["beta1"] @ arrs["Wv1"] + arrs["bv1"]
    flags = (
        bool(np.any(cq_full != 0)),
        bool(np.any(ck_full != 0)),
        bool(np.any(cv_full != 0)),
        bool(np.any(arrs["bse"] != 0)),
        bool(np.any(arrs["bv2"] != 0)),
    )
    nc = _get_nc(flags)

    in_maps = [_prep_core_inputs(c, arrs, flags) for c in range(NCORES)]

    from concourse.bass_utils import run_bass_kernel_spmd

    kw = {}
    prof_dir = os.environ.get("BASS_PROF_DIR")
    if prof_dir:
        os.makedirs(prof_dir, exist_ok=True)
        kw["tmpdir"] = prof_dir
    res = run_bass_kernel_spmd(nc, in_maps, list(range(NCORES)), **kw)
    last_results = res

    out = np.empty((BS, T, HID), np.float32)
    for c in range(NCORES):
        b, p = divmod(c, GROUP)
        o = np.asarray(res.results[c]["out"], np.float32)
        out[b, 256 * p : 256 * p + 256] = o[0:256]
        out[b, 1024 + 256 * p : 1024 + 256 * p + 256] = o[256:512]
    return out


# revision 15
# speedup vs baseline: 1.4520x; 1.0183x over previous
"""Trainium2 Bass kernel for nn_BasicAttentionBlock (8-core SPMD).

Math notes (validated against the reference in numpy first):

* The module is x + MHA1(LN(x)) + MHA2(LN(.), ctx) where ctx =
  relu(static @ Wse + bse) broadcast over time. Because every key/value row
  of attention-2 is identical per batch, softmax weights sum to 1 over equal
  value rows, so MHA2's output is exactly (ctx @ Wv2 + bv2) @ Wo2 broadcast
  over time — Q2/K2/scores2/softmax2 are skipped entirely.

* Attention-1 (16 heads, shared single V head, causal) is the real work.
  Sharding: batch b = core//4, and the 16 heads split 4-per-core within each
  batch group. The head-sum of attention vectors is reduce-scattered across
  the 4 cores of a batch group (token-sliced), and each core applies the
  out-projection + residual for its 512 tokens.

* LayerNorm is folded into the QKV projections: with W' = g⊙W,
  qT = rstd ⊙ (W'.T xT + (-colsum W')⊗m + (beta@W+b)⊗(1/rstd)).
  The two rank-1 corrections ride along as extra K=1 contraction rows; the
  per-token rstd scaling is one tensor_tensor multiply against a
  partition-broadcast rstd tile.

* Scores are computed transposed (S^T[key, q] on PE), exp on ScalarE with no
  max subtraction (|s| < ~4 by construction), causal handled by streaming
  only q >= key_block columns plus one 128x128 triangular mask multiply per
  diagonal block. p@v accumulates U^T[65, q] with a ones-column appended to
  V so row 64 collects the softmax denominators for free.
"""

import os
import sys

import numpy as np

if "/opt/trn_rl_repo" not in sys.path:
    sys.path.insert(0, "/opt/trn_rl_repo")

import ml_dtypes

BF = ml_dtypes.bfloat16

N_HEADS = 16
D_HEAD = 64
HID = 1024
T = 2048
BS = 2
LN_EPS = 1e-5
NCORES = 8
GROUP = 4  # cores per batch
HPC = N_HEADS // GROUP  # heads per core = 4
HCOLS = HPC * D_HEAD  # 256 projection cols per core
KT = HID // 128  # 8 k-tiles
NQT = T // 128  # 16 token tiles

_nc_cache = {}
last_results = None


def _build(flags):
    """Build the SPMD Bass program (same program for all 8 cores)."""
    has_cq, has_ck, has_cv, has_bse, has_bv2 = flags
    import concourse.bass as bass
    import concourse.tile as tile
    from concourse import bacc, mybir
    from concourse.masks import make_identity, make_upper_triangular

    f32 = mybir.dt.float32
    bf16 = mybir.dt.bfloat16
    AF = mybir.ActivationFunctionType
    ALU = mybir.AluOpType
    ts = bass.ts

    nc = bacc.Bacc("TRN2", target_bir_lowering=False)

    # ---- I/O ----
    xt_d = nc.dram_tensor("xt", [128, KT, T], bf16, kind="ExternalInput")
    xres_d = nc.dram_tensor("xres", [512, HID], f32, kind="ExternalInput")
    wq_d = nc.dram_tensor("wq", [128, KT, HCOLS], bf16, kind="ExternalInput")
    wk_d = nc.dram_tensor("wk", [128, KT, HCOLS], bf16, kind="ExternalInput")
    wv_d = nc.dram_tensor("wv", [128, KT, D_HEAD], bf16, kind="ExternalInput")
    sq_d = nc.dram_tensor("sq", [1, HCOLS], bf16, kind="ExternalInput")
    sk_d = nc.dram_tensor("sk", [1, HCOLS], bf16, kind="ExternalInput")
    sv_d = nc.dram_tensor("sv", [1, D_HEAD], bf16, kind="ExternalInput")
    cq_d = nc.dram_tensor("cq", [1, HCOLS], bf16, kind="ExternalInput") if has_cq else None
    ck_d = nc.dram_tensor("ck", [1, HCOLS], bf16, kind="ExternalInput") if has_ck else None
    cv_d = nc.dram_tensor("cv", [1, D_HEAD], bf16, kind="ExternalInput") if has_cv else None
    wo_d = nc.dram_tensor("wo", [D_HEAD, HID], bf16, kind="ExternalInput")
    sf_d = nc.dram_tensor("sf", [128, KT], bf16, kind="ExternalInput")
    wse_d = nc.dram_tensor("wse", [128, KT, HID], bf16, kind="ExternalInput")
    bse_d = nc.dram_tensor("bse", [1, HID], bf16, kind="ExternalInput") if has_bse else None
    wv2_d = nc.dram_tensor("wv2", [128, KT, D_HEAD], bf16, kind="ExternalInput")
    bv2_d = nc.dram_tensor("bv2", [1, D_HEAD], bf16, kind="ExternalInput") if has_bv2 else None
    wo2_d = nc.dram_tensor("wo2", [D_HEAD, HID], bf16, kind="ExternalInput")
    out_d = nc.dram_tensor("out", [512, HID], f32, kind="ExternalOutput")

    RG = [[0, 1, 2, 3], [4, 5, 6, 7]]

    with tile.TileContext(nc) as tc:
        with (
            tc.tile_pool(name="cpool", bufs=1) as cpool,
            tc.tile_pool(name="wpool", bufs=2) as wpool,
            tc.tile_pool(name="dpool", bufs=1, space="DRAM") as dpool,
        ):
            from contextlib import ExitStack

            _phase_a = ExitStack()
            ps_a = _phase_a.enter_context(tc.tile_pool(name="ps_a", bufs=3, space="PSUM"))
            ps_u = None  # opened for the attention phase after phase A closes
            def ctile(shape, dt, name):
                return cpool.tile(shape, dt, name=name, tag=name)

            def wtile(shape, dt, name, tag, bufs):
                return wpool.tile(shape, dt, name=name, tag=tag, bufs=bufs)

            def patile(shape, name):
                return ps_a.tile(shape, f32, name=name, tag="a")

            def putile(shape, dt, name):
                return ps_u.tile(shape, dt, name=name, tag="u")

            def pstile(shape, dt, name):
                return ps_s.tile(shape, dt, name=name, tag="s")

            # ---- constants / persistent SBUF ----
            xt_sb = ctile([128, KT, T], bf16, "xt_sb")
            for k in range(KT):
                nc.sync.dma_start(out=xt_sb[:, k, :], in_=xt_d[:, k, :])
            wq_sb = ctile([128, KT, HCOLS], bf16, "wq_sb")
            nc.sync.dma_start(out=wq_sb, in_=wq_d[:, :, :])
            wk_sb = ctile([128, KT, HCOLS], bf16, "wk_sb")
            nc.sync.dma_start(out=wk_sb, in_=wk_d[:, :, :])
            wv_sb = ctile([128, KT, D_HEAD], bf16, "wv_sb")
            nc.sync.dma_start(out=wv_sb, in_=wv_d[:, :, :])
            sq_sb = ctile([1, HCOLS], bf16, "sq_sb")
            nc.sync.dma_start(out=sq_sb, in_=sq_d[:, :])
            sk_sb = ctile([1, HCOLS], bf16, "sk_sb")
            nc.sync.dma_start(out=sk_sb, in_=sk_d[:, :])
            sv_sb = ctile([1, D_HEAD], bf16, "sv_sb")
            nc.sync.dma_start(out=sv_sb, in_=sv_d[:, :])
            cvec_sb = {}
            for nm, d in (("cq", cq_d), ("ck", ck_d), ("cv", cv_d)):
                if d is not None:
                    t_ = ctile([1, d.shape[1]], bf16, nm + "_sb")
                    nc.sync.dma_start(out=t_, in_=d[:, :])
                    cvec_sb[nm] = t_
            sf_sb = ctile([128, KT], bf16, "sf_sb")
            nc.sync.dma_start(out=sf_sb, in_=sf_d[:, :])
            wse_sb = ctile([128, KT, HID], bf16, "wse_sb")
            nc.sync.dma_start(out=wse_sb, in_=wse_d[:, :, :])
            if has_bse:
                bse_sb = ctile([1, HID], bf16, "bse_sb")
                nc.sync.dma_start(out=bse_sb, in_=bse_d[:, :])
            wv2_sb = ctile([128, KT, D_HEAD], bf16, "wv2_sb")
            nc.sync.dma_start(out=wv2_sb, in_=wv2_d[:, :, :])
            if has_bv2:
                bv2_sb = ctile([1, D_HEAD], bf16, "bv2_sb")
                nc.sync.dma_start(out=bv2_sb, in_=bv2_d[:, :])
            wo2_sb = ctile([D_HEAD, HID], bf16, "wo2_sb")
            nc.sync.dma_start(out=wo2_sb, in_=wo2_d[:, :])
            w_comb = ctile([D_HEAD + 1, HID], bf16, "w_comb")
            nc.sync.dma_start(out=w_comb[0:D_HEAD, :], in_=wo_d[:, :])
            xres_sb = ctile([128, 4, HID], f32, "xres_sb")
            for t_ in range(4):
                nc.sync.dma_start(out=xres_sb[:, t_, :], in_=xres_d[ts(t_, 128), :])

            ident = ctile([128, 128], bf16, "ident")
            make_identity(nc, ident)
            utri = ctile([128, 128], bf16, "utri")
            make_upper_triangular(nc, utri, val=1.0, diag=True)
            onesK = ctile([128, 1], bf16, "onesK")
            nc.vector.memset(onesK, 1.0 / HID)
            ones1 = ctile([1, 1], bf16, "ones1")
            nc.vector.memset(ones1, 1.0)
            eps_sb = ctile([1, 1], f32, "eps_sb")
            nc.vector.memset(eps_sb, LN_EPS)

            vrows = ctile([128, T], f32, "vrows")  # p0=rstd p32=m2 p64=var p96=lnv
            m_bf = ctile([1, T], bf16, "m_bf")
            invr_bf = ctile([1, T], bf16, "invr_bf")
            RSTD = ctile([128, T], f32, "RSTD")
            qT = [ctile([128, T], bf16, f"qT{i}") for i in range(2)]
            kT = [ctile([128, T], bf16, f"kT{i}") for i in range(2)]
            vT = ctile([D_HEAD, T], bf16, "vT")
            v_aug = ctile([128, NQT, D_HEAD + 1], bf16, "v_aug")
            nc.vector.memset(v_aug, 1.0)
            MT_acc = ctile([D_HEAD, T], f32, "MT_acc")
            MT_aug = ctile([D_HEAD + 1, 4, 128], bf16, "MT_aug")
            nc.vector.memset(MT_aug[D_HEAD : D_HEAD + 1, :, :], 1.0)
            ctxT_sb = ctile([128, KT], bf16, "ctxT_sb")

            # ---- static path: so2 = (relu(sf@Wse+bse) @ Wv2 + bv2) @ Wo2 ----
            ctx_ps = patile([1, HID], "ctx_ps")
            for n in range(2):
                c0 = n * 512
                for k in range(KT):
                    nc.tensor.matmul(
                        ctx_ps[0:1, c0 : c0 + 512],
                        sf_sb[:, k : k + 1],
                        wse_sb[:, k, c0 : c0 + 512],
                        start=(k == 0),
                        stop=(k == KT - 1 and not has_bse),
                    )
                if has_bse:
                    nc.tensor.matmul(
                        ctx_ps[0:1, c0 : c0 + 512],
                        ones1,
                        bse_sb[0:1, c0 : c0 + 512],
                        start=False,
                        stop=True,
                    )
            ctx_sb = ctile([1, HID], bf16, "ctx_sb")
            nc.scalar.activation(ctx_sb, ctx_ps, AF.Relu)
            ctx_bounce = dpool.tile([1, HID], bf16, name="ctx_bounce", tag="ctx_bounce")
            nc.gpsimd.dma_start(out=ctx_bounce[:, :], in_=ctx_sb[:, :])
            nc.gpsimd.dma_start(
                out=ctxT_sb[:, :], in_=ctx_bounce[0, :].rearrange("(k p) -> p k", p=128)
            )
            v2_ps = ps_a.tile([1, D_HEAD], f32, name="v2_ps", tag="a")
            for k in range(KT):
                nc.tensor.matmul(
                    v2_ps,
                    ctxT_sb[:, k : k + 1],
                    wv2_sb[:, k, :],
                    start=(k == 0),
                    stop=(k == KT - 1 and not has_bv2),
                )
            if has_bv2:
                nc.tensor.matmul(v2_ps, ones1, bv2_sb[0:1, :], start=False, stop=True)
            v2_sb = ctile([1, D_HEAD], bf16, "v2_sb")
            nc.scalar.activation(v2_sb, v2_ps, AF.Copy)
            v2T_ps = ps_a.tile([D_HEAD, 1], bf16, name="v2T_ps", tag="a")
            nc.tensor.transpose(v2T_ps, v2_sb[0:1, :], ident[0:1, 0:1])
            v2T_sb = ctile([D_HEAD, 1], bf16, "v2T_sb")
            nc.vector.tensor_copy(v2T_sb, v2T_ps)
            so2_ps = patile([1, HID], "so2_ps")
            for n in range(2):
                c0 = n * 512
                nc.tensor.matmul(
                    so2_ps[0:1, c0 : c0 + 512], v2T_sb, wo2_sb[:, c0 : c0 + 512],
                    start=True, stop=True,
                )
            nc.scalar.activation(w_comb[D_HEAD : D_HEAD + 1, :], so2_ps, AF.Copy)

            # ---- LN stats: m, E[x^2] via ones-matmuls over xT ----
            for half in range(2):
                hc0 = half * 1024
                m_ps = patile([1, 1024], f"m_ps{half}")
                ms_ps = patile([1, 1024], f"ms_ps{half}")
                for k in range(KT):
                    xsq = wtile([128, 1024], bf16, f"xsq{half}_{k}", "xsq", 2)
                    nc.vector.tensor_mul(
                        xsq, xt_sb[:, k, hc0 : hc0 + 1024], xt_sb[:, k, hc0 : hc0 + 1024]
                    )
                    for n in range(2):
                        c0 = n * 512
                        nc.tensor.matmul(
                            m_ps[0:1, c0 : c0 + 512],
                            onesK,
                            xt_sb[:, k, hc0 + c0 : hc0 + c0 + 512],
                            start=(k == 0),
                            stop=(k == KT - 1),
                        )
                        nc.tensor.matmul(
                            ms_ps[0:1, c0 : c0 + 512],
                            onesK,
                            xsq[:, c0 : c0 + 512],
                            start=(k == 0),
                            stop=(k == KT - 1),
                        )
                nc.scalar.activation(m_bf[0:1, hc0 : hc0 + 1024], m_ps, AF.Copy)
                nc.scalar.activation(vrows[32:33, hc0 : hc0 + 1024], m_ps, AF.Square)
                nc.vector.tensor_sub(
                    vrows[64:65, hc0 : hc0 + 1024], ms_ps, vrows[32:33, hc0 : hc0 + 1024]
                )
            nc.scalar.activation(vrows[96:97, :], vrows[64:65, :], AF.Ln, bias=eps_sb[0:1, 0:1])
            nc.scalar.activation(vrows[0:1, :], vrows[96:97, :], AF.Exp, scale=-0.5)
            nc.scalar.activation(invr_bf[0:1, :], vrows[96:97, :], AF.Exp, scale=0.5)
            nc.gpsimd.partition_broadcast(RSTD, vrows[0:1, :])

            # ---- projections qT/kT/vT (transposed, LN folded) ----
            projs = [
                ("q", wq_sb, sq_sb, cvec_sb.get("cq"), [qT[0], qT[1]], 128),
                ("k", wk_sb, sk_sb, cvec_sb.get("ck"), [kT[0], kT[1]], 128),
                ("v", wv_sb, sv_sb, cvec_sb.get("cv"), [vT], 64),
            ]
            for nm, w_sb, s_sb, c_sb, dests, P in projs:
                for mc, dest in enumerate(dests):
                    mcols = slice(mc * 128, mc * 128 + P)
                    for half in range(2):
                        hc0 = half * 1024
                        pp = patile([P, 1024], f"pp_{nm}{mc}{half}")
                        for n in range(2):
                            c0 = n * 512
                            for k in range(KT):
                                nc.tensor.matmul(
                                    pp[:, c0 : c0 + 512],
                                    w_sb[:, k, mcols],
                                    xt_sb[:, k, hc0 + c0 : hc0 + c0 + 512],
                                    start=(k == 0),
                                    stop=False,
                                )
                            nc.tensor.matmul(
                                pp[:, c0 : c0 + 512],
                                s_sb[0:1, mcols],
                                m_bf[0:1, hc0 + c0 : hc0 + c0 + 512],
                                start=False,
                                stop=(c_sb is None),
                            )
                            if c_sb is not None:
                                nc.tensor.matmul(
                                    pp[:, c0 : c0 + 512],
                                    c_sb[0:1, mcols],
                                    invr_bf[0:1, hc0 + c0 : hc0 + c0 + 512],
                                    start=False,
                                    stop=True,
                                )
                        nc.vector.tensor_mul(
                            dest[:P, hc0 : hc0 + 1024], pp, RSTD[:P, hc0 : hc0 + 1024]
                        )

            # ---- v_aug = [v | 1] in natural layout via PE transposes ----
            for t_ in range(NQT):
                vt_ps = ps_a.tile([128, D_HEAD], bf16, name=f"vt_ps{t_}", tag="a")
                nc.tensor.transpose(vt_ps, vT[0:D_HEAD, ts(t_, 128)], ident[0:D_HEAD, 0:D_HEAD])
                nc.vector.tensor_copy(v_aug[:, t_, 0:D_HEAD], vt_ps)

            # ---- attention (phase B): 512-col q-chunks, head-pair row-packed ----
            _phase_a.close()
            _phase_b = ExitStack()
            ps_s = _phase_b.enter_context(tc.tile_pool(name="ps_s", bufs=2, space="PSUM"))
            ps_u = _phase_b.enter_context(tc.tile_pool(name="ps_u", bufs=4, space="PSUM"))

            def launch_rs(c):
                cbase = 512 * c
                cc_in = dpool.tile([4, D_HEAD, 128], f32, name=f"cc_in{c}", tag=f"cc_in{c}")
                for blk in range(4):
                    nc.gpsimd.dma_start(
                        out=cc_in[blk],
                        in_=MT_acc[:, cbase + 128 * blk : cbase + 128 * blk + 128],
                    )
                cc_out = dpool.tile([D_HEAD, 128], f32, name=f"cc_out{c}", tag=f"cc_out{c}")
                nc.gpsimd.collective_compute(
                    "ReduceScatter",
                    ALU.add,
                    replica_groups=RG,
                    ins=[cc_in.opt()],
                    outs=[cc_out.opt()],
                )
                mt_st = wtile([D_HEAD, 128], f32, f"mt_st{c}", "mt_st", 2)
                nc.sync.dma_start(out=mt_st, in_=cc_out[:, :])
                nc.vector.tensor_copy(MT_aug[0:D_HEAD, c, :], mt_st)

            def do_outproj(c):
                out_sb = wtile([128, HID], f32, f"out_sb{c}", "out_sb", 2)
                for n in range(2):
                    c0 = n * 512
                    o_ps = putile([128, 512], f32, f"o_ps{c}{n}")
                    nc.tensor.matmul(
                        o_ps,
                        MT_aug[:, c, :],
                        w_comb[:, c0 : c0 + 512],
                        start=True,
                        stop=True,
                    )
                    nc.vector.tensor_add(
                        out_sb[:, c0 : c0 + 512], o_ps, xres_sb[:, c, c0 : c0 + 512]
                    )
                nc.sync.dma_start(out=out_d[ts(c, 128), :], in_=out_sb)

            for c in range(4):
                cbase = 512 * c
                for hp in range(2):
                    U = [
                        [putile([D_HEAD + 1, 512], f32, f"U{c}{hp}{hb}{s}") for s in range(1)]
                        for hb in range(2)
                    ]
                    jmax = 4 * c + 4
                    for j in range(jmax):
                        qs = max(cbase, j * 128)
                        N = cbase + 512 - qs
                        Sp = pstile([128, 1024], f32, f"S{c}{hp}{j}")
                        for hb in range(2):
                            rb = hb * 64
                            nc.tensor.matmul(
                                Sp[:, hb * 512 : hb * 512 + N],
                                kT[hp][rb : rb + 64, ts(j, 128)],
                                qT[hp][rb : rb + 64, qs : qs + N],
                                start=True,
                                stop=True,
                            )
                        E = wtile([128, 1024], bf16, f"E{c}{hp}{j}", "E", 3)
                        if N == 512:
                            nc.scalar.activation(E, Sp, AF.Exp)
                        else:
                            nc.scalar.activation(E[:, 0:N], Sp[:, 0:N], AF.Exp)
                            nc.scalar.activation(
                                E[:, 512 : 512 + N], Sp[:, 512 : 512 + N], AF.Exp
                            )
                        if j >= 4 * c:
                            nc.vector.tensor_mul(E[:, 0:128], E[:, 0:128], utri)
                            nc.vector.tensor_mul(E[:, 512:640], E[:, 512:640], utri)
                        u0 = qs - cbase
                        j_last = jmax - 1
                        for hb in range(2):
                            nc.tensor.matmul(
                                U[hb][0][:, u0 : u0 + N],
                                v_aug[:, j, :],
                                E[:, hb * 512 : hb * 512 + N],
                                start=(j == 0),
                                stop=(j == j_last),
                                skip_group_check=True,
                            )
                    for hb in range(2):
                        first_head = hp == 0 and hb == 0
                        gs = slice(cbase, cbase + 512)
                        dnr = wtile([1, 512], f32, f"dnr{c}{hp}{hb}", "dnr", 2)
                        nc.vector.tensor_copy(dnr, U[hb][0][D_HEAD : D_HEAD + 1, :])
                        dn = wtile([1, 512], f32, f"dn{c}{hp}{hb}", "dn", 2)
                        nc.vector.reciprocal_approx_fast(dn, dnr)
                        bc = wtile([64, 512], f32, f"bc{c}{hp}{hb}", "bc", 2)
                        nc.gpsimd.partition_broadcast(bc, dn[0:1, :])
                        if first_head:
                            nc.vector.tensor_mul(MT_acc[:, gs], U[hb][0][0:D_HEAD, :], bc)
                        else:
                            tmp = wtile([64, 512], f32, f"tmp{c}{hp}{hb}", "tmp", 2)
                            nc.vector.tensor_mul(tmp, U[hb][0][0:D_HEAD, :], bc)
                            nc.vector.tensor_add(MT_acc[:, gs], MT_acc[:, gs], tmp)
                launch_rs(c)
                if c >= 2:
                    do_outproj(c - 2)
            do_outproj(2)
            do_outproj(3)
            _phase_b.close()

    nc.compile()
    return nc


def _get_nc(flags):
    if flags not in _nc_cache:
        _nc_cache[flags] = _build(flags)
    return _nc_cache[flags]


def _prep_core_inputs(c, arrs, flags):
    has_cq, has_ck, has_cv, has_bse, has_bv2 = flags
    b, p = divmod(c, GROUP)
    x = arrs["x"][b]  # [T, HID] f32
    g1 = arrs["g1"]
    hs = slice(p * HCOLS, (p + 1) * HCOLS)

    def kmajor(w):  # [HID, C] -> [128, KT, C]
        return np.ascontiguousarray(
            w.reshape(KT, 128, w.shape[1]).transpose(1, 0, 2)
        ).astype(BF)

    wq_full = (g1[:, None] * arrs["Wq1"]) / 8.0
    wk_full = g1[:, None] * arrs["Wk1"]
    wv_full = g1[:, None] * arrs["Wv1"]
    wq = wq_full[:, hs]
    wk = wk_full[:, hs]
    cq_full = (arrs["beta1"] @ arrs["Wq1"] + arrs["bq1"]) / 8.0
    ck_full = arrs["beta1"] @ arrs["Wk1"] + arrs["bk1"]
    cv_full = arrs["beta1"] @ arrs["Wv1"] + arrs["bv1"]

    rows = np.r_[tuple(slice(512 * c + 128 * p, 512 * c + 128 * p + 128) for c in range(4))]
    d = {
        "xt": np.ascontiguousarray(
            x.T.reshape(KT, 128, T).transpose(1, 0, 2)
        ).astype(BF),
        "xres": np.ascontiguousarray(x[rows]).astype(np.float32),
        "wq": kmajor(wq),
        "wk": kmajor(wk),
        "wv": kmajor(wv_full),
        "sq": (-wq.astype(np.float64).sum(0))[None].astype(BF),
        "sk": (-wk.astype(np.float64).sum(0))[None].astype(BF),
        "sv": (-wv_full.astype(np.float64).sum(0))[None].astype(BF),
        "wo": (arrs["Wo1"] / float(N_HEADS)).astype(BF),
        "sf": np.ascontiguousarray(arrs["static_features"][b].reshape(KT, 128).T).astype(BF),
        "wse": kmajor(arrs["Wse"]),
        "wv2": kmajor(arrs["Wv2"]),
        "wo2": arrs["Wo2"].astype(BF),
    }
    if has_cq:
        d["cq"] = cq_full[hs][None].astype(BF)
    if has_ck:
        d["ck"] = ck_full[hs][None].astype(BF)
    if has_cv:
        d["cv"] = cv_full[None].astype(BF)
    if has_bse:
        d["bse"] = arrs["bse"][None].astype(BF)
    if has_bv2:
        d["bv2"] = arrs["bv2"][None].astype(BF)
    return d


def kernel(**inputs):
    global last_results
    arrs = {k: np.asarray(v, np.float32) for k, v in inputs.items()}

    cq_full = (arrs["beta1"] @ arrs["Wq1"] + arrs["bq1"]) / 8.0
    ck_full = arrs["beta1"] @ arrs["Wk1"] + arrs["bk1"]
    cv_full = arrs["beta1"] @ arrs["Wv1"] + arrs["bv1"]
    flags = (
        bool(np.any(cq_full != 0)),
        bool(np.any(ck_full != 0)),
        bool(np.any(cv_full != 0)),
        bool(np.any(arrs["bse"] != 0)),
        bool(np.any(arrs["bv2"] != 0)),
    )
    nc = _get_nc(flags)

    in_maps = [_prep_core_inputs(c, arrs, flags) for c in range(NCORES)]

    from concourse.bass_utils import run_bass_kernel_spmd

    kw = {}
    prof_dir = os.environ.get("BASS_PROF_DIR")
    if prof_dir:
        os.makedirs(prof_dir, exist_ok=True)
        kw["tmpdir"] = prof_dir
    res = run_bass_kernel_spmd(nc, in_maps, list(range(NCORES)), **kw)
    last_results = res

    out = np.empty((BS, T, HID), np.float32)
    for core in range(NCORES):
        b, p = divmod(core, GROUP)
        o = np.asarray(res.results[core]["out"], np.float32)
        for c in range(4):
            out[b, 512 * c + 128 * p : 512 * c + 128 * p + 128] = o[128 * c : 128 * c + 128]
    return out


# revision 17
# speedup vs baseline: 1.5032x; 1.0353x over previous
"""Trainium2 Bass kernel for nn_BasicAttentionBlock (8-core SPMD).

Math notes (validated against the reference in numpy first):

* The module is x + MHA1(LN(x)) + MHA2(LN(.), ctx) where ctx =
  relu(static @ Wse + bse) broadcast over time. Because every key/value row
  of attention-2 is identical per batch, softmax weights sum to 1 over equal
  value rows, so MHA2's output is exactly (ctx @ Wv2 + bv2) @ Wo2 broadcast
  over time — Q2/K2/scores2/softmax2 are skipped entirely.

* Attention-1 (16 heads, shared single V head, causal) is the real work.
  Sharding: batch b = core//4, and the 16 heads split 4-per-core within each
  batch group. The head-sum of attention vectors is reduce-scattered across
  the 4 cores of a batch group (token-sliced), and each core applies the
  out-projection + residual for its 512 tokens.

* LayerNorm is folded into the QKV projections: with W' = g⊙W,
  qT = rstd ⊙ (W'.T xT + (-colsum W')⊗m + (beta@W+b)⊗(1/rstd)).
  The two rank-1 corrections ride along as extra K=1 contraction rows; the
  per-token rstd scaling is one tensor_tensor multiply against a
  partition-broadcast rstd tile.

* Scores are computed transposed (S^T[key, q] on PE), exp on ScalarE with no
  max subtraction (|s| < ~4 by construction), causal handled by streaming
  only q >= key_block columns plus one 128x128 triangular mask multiply per
  diagonal block. p@v accumulates U^T[65, q] with a ones-column appended to
  V so row 64 collects the softmax denominators for free.
"""

import os
import sys

import numpy as np

if "/opt/trn_rl_repo" not in sys.path:
    sys.path.insert(0, "/opt/trn_rl_repo")

import ml_dtypes

BF = ml_dtypes.bfloat16

N_HEADS = 16
D_HEAD = 64
HID = 1024
T = 2048
BS = 2
LN_EPS = 1e-5
NCORES = 8
GROUP = 4  # cores per batch
HPC = N_HEADS // GROUP  # heads per core = 4
HCOLS = HPC * D_HEAD  # 256 projection cols per core
KT = HID // 128  # 8 k-tiles
NQT = T // 128  # 16 token tiles

_nc_cache = {}
last_results = None

IMG_SEGS = [("sf", 8), ("wq", 2048), ("wk", 2048), ("wv", 512), ("wse", 8192),
            ("wv2", 512), ("wo2", 1024), ("wo", 1024)]
IMG_COLS = sum(c for _, c in IMG_SEGS)
IMG_OFF = {}
_o = 0
for _n, _c in IMG_SEGS:
    IMG_OFF[_n] = _o
    _o += _c


def _vrow_layout(flags):
    has_cq, has_ck, has_cv, has_bse, has_bv2 = flags
    segs = [("sq", 256), ("sk", 256), ("sv", 64)]
    if has_cq:
        segs.append(("cq", 256))
    if has_ck:
        segs.append(("ck", 256))
    if has_cv:
        segs.append(("cv", 64))
    if has_bse:
        segs.append(("bse", 1024))
    if has_bv2:
        segs.append(("bv2", 64))
    off = {}
    o = 0
    for n, c in segs:
        off[n] = (o, c)
        o += c
    return off, max(o, 64)


def _build(flags):
    """Build the SPMD Bass program (same program for all 8 cores)."""
    has_cq, has_ck, has_cv, has_bse, has_bv2 = flags
    import concourse.bass as bass
    import concourse.tile as tile
    from concourse import bacc, mybir
    from concourse.masks import make_identity, make_upper_triangular

    f32 = mybir.dt.float32
    bf16 = mybir.dt.bfloat16
    AF = mybir.ActivationFunctionType
    ALU = mybir.AluOpType
    ts = bass.ts

    nc = bacc.Bacc("TRN2", target_bir_lowering=False)

    # ---- I/O ----
    voff, vcols = _vrow_layout(flags)
    xt_d = nc.dram_tensor("xt", [128, KT, T], bf16, kind="ExternalInput")
    xres_d = nc.dram_tensor("xres", [512, HID], f32, kind="ExternalInput")
    img_d = nc.dram_tensor("img", [128, IMG_COLS], bf16, kind="ExternalInput")
    vrow_d = nc.dram_tensor("vrow", [1, vcols], bf16, kind="ExternalInput")
    out_d = nc.dram_tensor("out", [512, HID], f32, kind="ExternalOutput")

    RG = [[0, 1, 2, 3], [4, 5, 6, 7]]

    with tile.TileContext(nc) as tc:
        with (
            tc.tile_pool(name="cpool", bufs=1) as cpool,
            tc.tile_pool(name="wpool", bufs=2) as wpool,
            tc.tile_pool(name="dpool", bufs=1, space="DRAM") as dpool,
        ):
            from contextlib import ExitStack

            _phase_a = ExitStack()
            ps_a = _phase_a.enter_context(tc.tile_pool(name="ps_a", bufs=3, space="PSUM"))
            ps_u = None  # opened for the attention phase after phase A closes
            def ctile(shape, dt, name):
                return cpool.tile(shape, dt, name=name, tag=name)

            def wtile(shape, dt, name, tag, bufs):
                return wpool.tile(shape, dt, name=name, tag=tag, bufs=bufs)

            def patile(shape, name):
                return ps_a.tile(shape, f32, name=name, tag="a")

            def putile(shape, dt, name):
                return ps_u.tile(shape, dt, name=name, tag="u")

            def pstile(shape, dt, name):
                return ps_s.tile(shape, dt, name=name, tag="s")

            # ---- constants / persistent SBUF ----
            xt_sb = ctile([128, KT, T], bf16, "xt_sb")
            nc.sync.dma_start(out=xt_sb[:, 0:4, :], in_=xt_d[:, 0:4, :])
            nc.sync.dma_start(out=xt_sb[:, 4:8, :], in_=xt_d[:, 4:8, :])
            img_sb = ctile([128, IMG_COLS], bf16, "img_sb")
            nc.sync.dma_start(out=img_sb, in_=img_d[:, :])
            vrow_sb = ctile([1, vcols], bf16, "vrow_sb")
            nc.sync.dma_start(out=vrow_sb, in_=vrow_d[:, :])
            xres_sb = ctile([128, 4, HID], f32, "xres_sb")
            for t_ in range(4):
                nc.gpsimd.dma_start(out=xres_sb[:, t_, :], in_=xres_d[ts(t_, 128), :])

            def iseg(name, k=None):
                o = IMG_OFF[name]
                if name in ("wq", "wk"):
                    return img_sb[:, o : o + 2048].rearrange("p (k c) -> p k c", k=KT)
                if name in ("wv", "wv2"):
                    return img_sb[:, o : o + 512].rearrange("p (k c) -> p k c", k=KT)
                if name == "wse":
                    return img_sb[:, o : o + 8192].rearrange("p (k c) -> p k c", k=KT)
                if name == "sf":
                    return img_sb[:, o : o + 8]
                return img_sb[:, o : o + 1024]

            def vseg(name):
                o, c = voff[name]
                return vrow_sb[0:1, o : o + c]

            sf_sb = iseg("sf")
            wq_sb = iseg("wq")
            wk_sb = iseg("wk")
            wv_sb = iseg("wv")
            wse_sb = iseg("wse")
            wv2_sb = iseg("wv2")
            wo2_sb = iseg("wo2")[0:D_HEAD, :]
            sq_sb = vseg("sq")
            sk_sb = vseg("sk")
            sv_sb = vseg("sv")
            cvec_sb = {}
            for nm, has in (("cq", has_cq), ("ck", has_ck), ("cv", has_cv)):
                if has:
                    cvec_sb[nm] = vseg(nm)
            bse_sb = vseg("bse") if has_bse else None
            bv2_sb = vseg("bv2") if has_bv2 else None
            w_comb = ctile([D_HEAD + 1, HID], bf16, "w_comb")
            nc.sync.dma_start(
                out=w_comb[0:D_HEAD, :], in_=img_sb[0:D_HEAD, IMG_OFF["wo"] : IMG_OFF["wo"] + 1024]
            )

            ident = ctile([128, 128], bf16, "ident")
            make_identity(nc, ident)
            utri = ctile([128, 128], bf16, "utri")
            make_upper_triangular(nc, utri, val=1.0, diag=True)
            onesK = ctile([128, 1], bf16, "onesK")
            nc.vector.memset(onesK, 1.0 / HID)
            ones1 = ctile([1, 1], bf16, "ones1")
            nc.vector.memset(ones1, 1.0)
            eps_sb = ctile([1, 1], f32, "eps_sb")
            nc.vector.memset(eps_sb, LN_EPS)

            vrows = ctile([128, T], f32, "vrows")  # p0=rstd p32=m2 p64=var p96=lnv
            m_bf = ctile([1, T], bf16, "m_bf")
            invr_bf = ctile([1, T], bf16, "invr_bf")
            RSTD = ctile([128, T], f32, "RSTD")
            qT = [ctile([128, T], bf16, f"qT{i}") for i in range(2)]
            kT = [ctile([128, T], bf16, f"kT{i}") for i in range(2)]
            vT = ctile([D_HEAD, T], bf16, "vT")
            v_aug = ctile([128, NQT, D_HEAD + 1], bf16, "v_aug")
            nc.vector.memset(v_aug, 1.0)
            MT_acc = ctile([D_HEAD, T], f32, "MT_acc")
            MT_aug = ctile([D_HEAD + 1, 4, 128], bf16, "MT_aug")
            nc.vector.memset(MT_aug[D_HEAD : D_HEAD + 1, :, :], 1.0)
            ctxT_sb = ctile([128, KT], bf16, "ctxT_sb")

            # ---- static path: so2 = (relu(sf@Wse+bse) @ Wv2 + bv2) @ Wo2 ----
            ctx_ps = patile([1, HID], "ctx_ps")
            for n in range(2):
                c0 = n * 512
                for k in range(KT):
                    nc.tensor.matmul(
                        ctx_ps[0:1, c0 : c0 + 512],
                        sf_sb[:, k : k + 1],
                        wse_sb[:, k, c0 : c0 + 512],
                        start=(k == 0),
                        stop=(k == KT - 1 and not has_bse),
                    )
                if has_bse:
                    nc.tensor.matmul(
                        ctx_ps[0:1, c0 : c0 + 512],
                        ones1,
                        bse_sb[:, c0 : c0 + 512],
                        start=False,
                        stop=True,
                    )
            ctx_sb = ctile([1, HID], bf16, "ctx_sb")
            nc.scalar.activation(ctx_sb, ctx_ps, AF.Relu)
            ctx_bounce = dpool.tile([1, HID], bf16, name="ctx_bounce", tag="ctx_bounce")
            nc.gpsimd.dma_start(out=ctx_bounce[:, :], in_=ctx_sb[:, :])
            nc.gpsimd.dma_start(
                out=ctxT_sb[:, :], in_=ctx_bounce[0, :].rearrange("(k p) -> p k", p=128)
            )
            v2_ps = ps_a.tile([1, D_HEAD], f32, name="v2_ps", tag="a")
            for k in range(KT):
                nc.tensor.matmul(
                    v2_ps,
                    ctxT_sb[:, k : k + 1],
                    wv2_sb[:, k, :],
                    start=(k == 0),
                    stop=(k == KT - 1 and not has_bv2),
                )
            if has_bv2:
                nc.tensor.matmul(v2_ps, ones1, bv2_sb, start=False, stop=True)
            v2_sb = ctile([1, D_HEAD], bf16, "v2_sb")
            nc.scalar.activation(v2_sb, v2_ps, AF.Copy)
            v2T_ps = ps_a.tile([D_HEAD, 1], bf16, name="v2T_ps", tag="a")
            nc.tensor.transpose(v2T_ps, v2_sb[0:1, :], ident[0:1, 0:1])
            v2T_sb = ctile([D_HEAD, 1], bf16, "v2T_sb")
            nc.vector.tensor_copy(v2T_sb, v2T_ps)
            so2_ps = patile([1, HID], "so2_ps")
            for n in range(2):
                c0 = n * 512
                nc.tensor.matmul(
                    so2_ps[0:1, c0 : c0 + 512], v2T_sb, wo2_sb[:, c0 : c0 + 512],
                    start=True, stop=True,
                )
            nc.scalar.activation(w_comb[D_HEAD : D_HEAD + 1, :], so2_ps, AF.Copy)

            # ---- LN stats: m, E[x^2] via ones-matmuls over xT ----
            for half in range(2):
                hc0 = half * 1024
                m_ps = patile([1, 1024], f"m_ps{half}")
                ms_ps = patile([1, 1024], f"ms_ps{half}")
                for k in range(KT):
                    xsq = wtile([128, 1024], bf16, f"xsq{half}_{k}", "xsq", 2)
                    nc.vector.tensor_mul(
                        xsq, xt_sb[:, k, hc0 : hc0 + 1024], xt_sb[:, k, hc0 : hc0 + 1024]
                    )
                    for n in range(2):
                        c0 = n * 512
                        nc.tensor.matmul(
                            m_ps[0:1, c0 : c0 + 512],
                            onesK,
                            xt_sb[:, k, hc0 + c0 : hc0 + c0 + 512],
                            start=(k == 0),
                            stop=(k == KT - 1),
                        )
                        nc.tensor.matmul(
                            ms_ps[0:1, c0 : c0 + 512],
                            onesK,
                            xsq[:, c0 : c0 + 512],
                            start=(k == 0),
                            stop=(k == KT - 1),
                        )
                nc.scalar.activation(m_bf[0:1, hc0 : hc0 + 1024], m_ps, AF.Copy)
                nc.scalar.activation(vrows[32:33, hc0 : hc0 + 1024], m_ps, AF.Square)
                nc.vector.tensor_sub(
                    vrows[64:65, hc0 : hc0 + 1024], ms_ps, vrows[32:33, hc0 : hc0 + 1024]
                )
            nc.scalar.activation(vrows[96:97, :], vrows[64:65, :], AF.Ln, bias=eps_sb[0:1, 0:1])
            nc.scalar.activation(vrows[0:1, :], vrows[96:97, :], AF.Exp, scale=-0.5)
            nc.scalar.activation(invr_bf[0:1, :], vrows[96:97, :], AF.Exp, scale=0.5)
            nc.gpsimd.partition_broadcast(RSTD, vrows[0:1, :])

            # ---- projections qT/kT/vT (transposed, LN folded) ----
            projs = [
                ("q", wq_sb, sq_sb, cvec_sb.get("cq"), [qT[0], qT[1]], 128),
                ("k", wk_sb, sk_sb, cvec_sb.get("ck"), [kT[0], kT[1]], 128),
                ("v", wv_sb, sv_sb, cvec_sb.get("cv"), [vT], 64),
            ]
            for nm, w_sb, s_sb, c_sb, dests, P in projs:
                for mc, dest in enumerate(dests):
                    mcols = slice(mc * 128, mc * 128 + P)
                    for half in range(2):
                        hc0 = half * 1024
                        pp = patile([P, 1024], f"pp_{nm}{mc}{half}")
                        for n in range(2):
                            c0 = n * 512
                            for k in range(KT):
                                nc.tensor.matmul(
                                    pp[:, c0 : c0 + 512],
                                    w_sb[:, k, mcols],
                                    xt_sb[:, k, hc0 + c0 : hc0 + c0 + 512],
                                    start=(k == 0),
                                    stop=False,
                                )
                            nc.tensor.matmul(
                                pp[:, c0 : c0 + 512],
                                s_sb[:, mcols],
                                m_bf[0:1, hc0 + c0 : hc0 + c0 + 512],
                                start=False,
                                stop=(c_sb is None),
                            )
                            if c_sb is not None:
                                nc.tensor.matmul(
                                    pp[:, c0 : c0 + 512],
                                    c_sb[:, mcols],
                                    invr_bf[0:1, hc0 + c0 : hc0 + c0 + 512],
                                    start=False,
                                    stop=True,
                                )
                        nc.vector.tensor_mul(
                            dest[:P, hc0 : hc0 + 1024], pp, RSTD[:P, hc0 : hc0 + 1024]
                        )

            # ---- v_aug = [v | 1] in natural layout via PE transposes ----
            for t_ in range(NQT):
                vt_ps = ps_a.tile([128, D_HEAD], bf16, name=f"vt_ps{t_}", tag="a")
                nc.tensor.transpose(vt_ps, vT[0:D_HEAD, ts(t_, 128)], ident[0:D_HEAD, 0:D_HEAD])
                nc.vector.tensor_copy(v_aug[:, t_, 0:D_HEAD], vt_ps)

            # ---- attention (phase B): 512-col q-chunks, head-pair row-packed ----
            _phase_a.close()
            _phase_b = ExitStack()
            ps_s = _phase_b.enter_context(tc.tile_pool(name="ps_s", bufs=2, space="PSUM"))
            ps_u = _phase_b.enter_context(tc.tile_pool(name="ps_u", bufs=4, space="PSUM"))

            def launch_rs(c):
                cbase = 512 * c
                cc_in = dpool.tile([4, D_HEAD, 128], f32, name=f"cc_in{c}", tag=f"cc_in{c}")
                for blk in range(4):
                    nc.gpsimd.dma_start(
                        out=cc_in[blk],
                        in_=MT_acc[:, cbase + 128 * blk : cbase + 128 * blk + 128],
                    )
                cc_out = dpool.tile([D_HEAD, 128], f32, name=f"cc_out{c}", tag=f"cc_out{c}")
                nc.gpsimd.collective_compute(
                    "ReduceScatter",
                    ALU.add,
                    replica_groups=RG,
                    ins=[cc_in.opt()],
                    outs=[cc_out.opt()],
                )
                mt_st = wtile([D_HEAD, 128], f32, f"mt_st{c}", "mt_st", 2)
                nc.sync.dma_start(out=mt_st, in_=cc_out[:, :])
                nc.vector.tensor_copy(MT_aug[0:D_HEAD, c, :], mt_st)

            def do_outproj(c):
                out_sb = wtile([128, HID], f32, f"out_sb{c}", "out_sb", 2)
                for n in range(2):
                    c0 = n * 512
                    o_ps = putile([128, 512], f32, f"o_ps{c}{n}")
                    nc.tensor.matmul(
                        o_ps,
                        MT_aug[:, c, :],
                        w_comb[:, c0 : c0 + 512],
                        start=True,
                        stop=True,
                    )
                    nc.vector.tensor_add(
                        out_sb[:, c0 : c0 + 512], o_ps, xres_sb[:, c, c0 : c0 + 512]
                    )
                nc.sync.dma_start(out=out_d[ts(c, 128), :], in_=out_sb)

            chunk_order = [3, 2, 1, 0]
            for ci, c in enumerate(chunk_order):
                cbase = 512 * c
                for hp in range(2):
                    U = [
                        [putile([D_HEAD + 1, 512], f32, f"U{c}{hp}{hb}{s}") for s in range(1)]
                        for hb in range(2)
                    ]
                    jmax = 4 * c + 4
                    for j in range(jmax):
                        qs = max(cbase, j * 128)
                        N = cbase + 512 - qs
                        Sp = pstile([128, 1024], f32, f"S{c}{hp}{j}")
                        for hb in range(2):
                            rb = hb * 64
                            nc.tensor.matmul(
                                Sp[:, hb * 512 : hb * 512 + N],
                                kT[hp][rb : rb + 64, ts(j, 128)],
                                qT[hp][rb : rb + 64, qs : qs + N],
                                start=True,
                                stop=True,
                            )
                        E = wtile([128, 1024], bf16, f"E{c}{hp}{j}", "E", 3)
                        if N == 512:
                            nc.scalar.activation(E, Sp, AF.Exp)
                        else:
                            nc.scalar.activation(E[:, 0:N], Sp[:, 0:N], AF.Exp)
                            nc.scalar.activation(
                                E[:, 512 : 512 + N], Sp[:, 512 : 512 + N], AF.Exp
                            )
                        if j >= 4 * c:
                            nc.vector.tensor_mul(E[:, 0:128], E[:, 0:128], utri)
                            nc.vector.tensor_mul(E[:, 512:640], E[:, 512:640], utri)
                        u0 = qs - cbase
                        j_last = jmax - 1
                        for hb in range(2):
                            nc.tensor.matmul(
                                U[hb][0][:, u0 : u0 + N],
                                v_aug[:, j, :],
                                E[:, hb * 512 : hb * 512 + N],
                                start=(j == 0),
                                stop=(j == j_last),
                                skip_group_check=True,
                            )
                    for hb in range(2):
                        first_head = hp == 0 and hb == 0
                        gs = slice(cbase, cbase + 512)
                        dnr = wtile([1, 512], f32, f"dnr{c}{hp}{hb}", "dnr", 2)
                        nc.vector.tensor_copy(dnr, U[hb][0][D_HEAD : D_HEAD + 1, :])
                        dn = wtile([1, 512], f32, f"dn{c}{hp}{hb}", "dn", 2)
                        nc.vector.reciprocal_approx_fast(dn, dnr)
                        bc = wtile([64, 512], f32, f"bc{c}{hp}{hb}", "bc", 2)
                        nc.gpsimd.partition_broadcast(bc, dn[0:1, :])
                        if first_head:
                            nc.vector.tensor_mul(MT_acc[:, gs], U[hb][0][0:D_HEAD, :], bc)
                        else:
                            tmp = wtile([64, 512], f32, f"tmp{c}{hp}{hb}", "tmp", 2)
                            nc.vector.tensor_mul(tmp, U[hb][0][0:D_HEAD, :], bc)
                            nc.vector.tensor_add(MT_acc[:, gs], MT_acc[:, gs], tmp)
                launch_rs(c)
                if ci >= 2:
                    do_outproj(chunk_order[ci - 2])
            do_outproj(chunk_order[2])
            do_outproj(chunk_order[3])
            _phase_b.close()

    nc.compile()
    return nc


def _get_nc(flags):
    if flags not in _nc_cache:
        _nc_cache[flags] = _build(flags)
    return _nc_cache[flags]


def _prep_core_inputs(c, arrs, flags):
    has_cq, has_ck, has_cv, has_bse, has_bv2 = flags
    b, p = divmod(c, GROUP)
    x = arrs["x"][b]  # [T, HID] f32
    g1 = arrs["g1"]
    hs = slice(p * HCOLS, (p + 1) * HCOLS)

    def kmajor(w):  # [HID, C] -> [128, KT*C] in k-major layout
        return np.ascontiguousarray(
            w.reshape(KT, 128, w.shape[1]).transpose(1, 0, 2).reshape(128, -1)
        )

    wq_full = (g1[:, None] * arrs["Wq1"]) / 8.0
    wk_full = g1[:, None] * arrs["Wk1"]
    wv_full = g1[:, None] * arrs["Wv1"]
    wq = wq_full[:, hs]
    wk = wk_full[:, hs]
    cq_full = (arrs["beta1"] @ arrs["Wq1"] + arrs["bq1"]) / 8.0
    ck_full = arrs["beta1"] @ arrs["Wk1"] + arrs["bk1"]
    cv_full = arrs["beta1"] @ arrs["Wv1"] + arrs["bv1"]

    img = np.zeros((128, IMG_COLS), np.float32)
    img[:, IMG_OFF["sf"] : IMG_OFF["sf"] + KT] = arrs["static_features"][b].reshape(KT, 128).T
    img[:, IMG_OFF["wq"] : IMG_OFF["wq"] + 2048] = kmajor(wq)
    img[:, IMG_OFF["wk"] : IMG_OFF["wk"] + 2048] = kmajor(wk)
    img[:, IMG_OFF["wv"] : IMG_OFF["wv"] + 512] = kmajor(wv_full)
    img[:, IMG_OFF["wse"] : IMG_OFF["wse"] + 8192] = kmajor(arrs["Wse"])
    img[:, IMG_OFF["wv2"] : IMG_OFF["wv2"] + 512] = kmajor(arrs["Wv2"])
    img[0:D_HEAD, IMG_OFF["wo2"] : IMG_OFF["wo2"] + 1024] = arrs["Wo2"]
    img[0:D_HEAD, IMG_OFF["wo"] : IMG_OFF["wo"] + 1024] = arrs["Wo1"] / float(N_HEADS)

    voff, vcols = _vrow_layout(flags)
    vrow = np.zeros((1, vcols), np.float32)

    def putv(name, vec):
        o, n = voff[name]
        vrow[0, o : o + len(vec)] = vec

    putv("sq", -wq.astype(np.float64).sum(0))
    putv("sk", -wk.astype(np.float64).sum(0))
    putv("sv", -wv_full.astype(np.float64).sum(0))
    if has_cq:
        putv("cq", cq_full[hs])
    if has_ck:
        putv("ck", ck_full[hs])
    if has_cv:
        putv("cv", cv_full)
    if has_bse:
        putv("bse", arrs["bse"])
    if has_bv2:
        putv("bv2", arrs["bv2"])

    rows = np.r_[tuple(slice(512 * c + 128 * p, 512 * c + 128 * p + 128) for c in range(4))]
    return {
        "xt": np.ascontiguousarray(
            x.T.reshape(KT, 128, T).transpose(1, 0, 2)
        ).astype(BF),
        "xres": np.ascontiguousarray(x[rows]).astype(np.float32),
        "img": img.astype(BF),
        "vrow": vrow.astype(BF),
    }


def kernel(**inputs):
    global last_results
    arrs = {k: np.asarray(v, np.float32) for k, v in inputs.items()}

    cq_full = (arrs["beta1"] @ arrs["Wq1"] + arrs["bq1"]) / 8.0
    ck_full = arrs["beta1"] @ arrs["Wk1"] + arrs["bk1"]
    cv_full = arrs["beta1"] @ arrs["Wv1"] + arrs["bv1"]
    flags = (
        bool(np.any(cq_full != 0)),
        bool(np.any(ck_full != 0)),
        bool(np.any(cv_full != 0)),
        bool(np.any(arrs["bse"] != 0)),
        bool(np.any(arrs["bv2"] != 0)),
    )
    nc = _get_nc(flags)

    in_maps = [_prep_core_inputs(c, arrs, flags) for c in range(NCORES)]

    from concourse.bass_utils import run_bass_kernel_spmd

    kw = {}
    prof_dir = os.environ.get("BASS_PROF_DIR")
    if prof_dir:
        os.makedirs(prof_dir, exist_ok=True)
        kw["tmpdir"] = prof_dir
    res = run_bass_kernel_spmd(nc, in_maps, list(range(NCORES)), **kw)
    last_results = res

    out = np.empty((BS, T, HID), np.float32)
    for core in range(NCORES):
        b, p = divmod(core, GROUP)
        o = np.asarray(res.results[core]["out"], np.float32)
        for c in range(4):
            out[b, 512 * c + 128 * p : 512 * c + 128 * p + 128] = o[128 * c : 128 * c + 128]
    return out


# revision 18
# speedup vs baseline: 1.5537x; 1.0335x over previous
"""Trainium2 Bass kernel for nn_BasicAttentionBlock (8-core SPMD).

Math notes (validated against the reference in numpy first):

* The module is x + MHA1(LN(x)) + MHA2(LN(.), ctx) where ctx =
  relu(static @ Wse + bse) broadcast over time. Because every key/value row
  of attention-2 is identical per batch, softmax weights sum to 1 over equal
  value rows, so MHA2's output is exactly (ctx @ Wv2 + bv2) @ Wo2 broadcast
  over time — Q2/K2/scores2/softmax2 are skipped entirely.

* Attention-1 (16 heads, shared single V head, causal) is the real work.
  Sharding: batch b = core//4, and the 16 heads split 4-per-core within each
  batch group. The head-sum of attention vectors is reduce-scattered across
  the 4 cores of a batch group (token-sliced), and each core applies the
  out-projection + residual for its 512 tokens.

* LayerNorm is folded into the QKV projections: with W' = g⊙W,
  qT = rstd ⊙ (W'.T xT + (-colsum W')⊗m + (beta@W+b)⊗(1/rstd)).
  The two rank-1 corrections ride along as extra K=1 contraction rows; the
  per-token rstd scaling is one tensor_tensor multiply against a
  partition-broadcast rstd tile.

* Scores are computed transposed (S^T[key, q] on PE), exp on ScalarE with no
  max subtraction (|s| < ~4 by construction), causal handled by streaming
  only q >= key_block columns plus one 128x128 triangular mask multiply per
  diagonal block. p@v accumulates U^T[65, q] with a ones-column appended to
  V so row 64 collects the softmax denominators for free.
"""

import os
import sys

import numpy as np

if "/opt/trn_rl_repo" not in sys.path:
    sys.path.insert(0, "/opt/trn_rl_repo")

import ml_dtypes

BF = ml_dtypes.bfloat16

N_HEADS = 16
D_HEAD = 64
HID = 1024
T = 2048
BS = 2
LN_EPS = 1e-5
NCORES = 8
GROUP = 4  # cores per batch
HPC = N_HEADS // GROUP  # heads per core = 4
HCOLS = HPC * D_HEAD  # 256 projection cols per core
KT = HID // 128  # 8 k-tiles
NQT = T // 128  # 16 token tiles

_nc_cache = {}
last_results = None

IMG_A_SEGS = [("wq", 2048), ("wk", 2048), ("wv", 512)]
IMG_B_SEGS = [("sf", 8), ("wse", 8192), ("wv2", 512), ("wo2", 1024), ("wo", 1024)]
IMG_A_COLS = sum(c for _, c in IMG_A_SEGS)
IMG_B_COLS = sum(c for _, c in IMG_B_SEGS)
IMG_OFF = {}
_o = 0
for _n, _c in IMG_A_SEGS:
    IMG_OFF[_n] = _o
    _o += _c
_o = 0
for _n, _c in IMG_B_SEGS:
    IMG_OFF[_n] = _o
    _o += _c


def _vrow_layout(flags):
    has_cq, has_ck, has_cv, has_bse, has_bv2 = flags
    segs = [("sq", 256), ("sk", 256), ("sv", 64)]
    if has_cq:
        segs.append(("cq", 256))
    if has_ck:
        segs.append(("ck", 256))
    if has_cv:
        segs.append(("cv", 64))
    if has_bse:
        segs.append(("bse", 1024))
    if has_bv2:
        segs.append(("bv2", 64))
    off = {}
    o = 0
    for n, c in segs:
        off[n] = (o, c)
        o += c
    return off, max(o, 64)


def _build(flags):
    """Build the SPMD Bass program (same program for all 8 cores)."""
    has_cq, has_ck, has_cv, has_bse, has_bv2 = flags
    import concourse.bass as bass
    import concourse.tile as tile
    from concourse import bacc, mybir
    from concourse.masks import make_identity, make_upper_triangular

    f32 = mybir.dt.float32
    bf16 = mybir.dt.bfloat16
    AF = mybir.ActivationFunctionType
    ALU = mybir.AluOpType
    ts = bass.ts

    nc = bacc.Bacc("TRN2", target_bir_lowering=False)

    # ---- I/O ----
    voff, vcols = _vrow_layout(flags)
    xt_d = nc.dram_tensor("xt", [128, KT, T], bf16, kind="ExternalInput")
    xres_d = nc.dram_tensor("xres", [512, HID], f32, kind="ExternalInput")
    imga_d = nc.dram_tensor("imga", [128, IMG_A_COLS], bf16, kind="ExternalInput")
    imgb_d = nc.dram_tensor("imgb", [128, IMG_B_COLS], bf16, kind="ExternalInput")
    vrow_d = nc.dram_tensor("vrow", [1, vcols], bf16, kind="ExternalInput")
    out_d = nc.dram_tensor("out", [512, HID], f32, kind="ExternalOutput")

    RG = [[0, 1, 2, 3], [4, 5, 6, 7]]

    with tile.TileContext(nc) as tc:
        with (
            tc.tile_pool(name="cpool", bufs=1) as cpool,
            tc.tile_pool(name="wpool", bufs=2) as wpool,
            tc.tile_pool(name="dpool", bufs=1, space="DRAM") as dpool,
        ):
            from contextlib import ExitStack

            _phase_a = ExitStack()
            ps_a = _phase_a.enter_context(tc.tile_pool(name="ps_a", bufs=3, space="PSUM"))
            ps_u = None  # opened for the attention phase after phase A closes
            def ctile(shape, dt, name):
                return cpool.tile(shape, dt, name=name, tag=name)

            def wtile(shape, dt, name, tag, bufs):
                return wpool.tile(shape, dt, name=name, tag=tag, bufs=bufs)

            def patile(shape, name):
                return ps_a.tile(shape, f32, name=name, tag="a")

            def putile(shape, dt, name):
                return ps_u.tile(shape, dt, name=name, tag="u")

            def pstile(shape, dt, name):
                return ps_s.tile(shape, dt, name=name, tag="s")

            # ---- constants / persistent SBUF ----
            xt_sb = ctile([128, KT, T], bf16, "xt_sb")
            nc.sync.dma_start(out=xt_sb[:, 0:4, :], in_=xt_d[:, 0:4, :])
            nc.sync.dma_start(out=xt_sb[:, 4:8, :], in_=xt_d[:, 4:8, :])
            imga_sb = ctile([128, IMG_A_COLS], bf16, "imga_sb")
            nc.sync.dma_start(out=imga_sb, in_=imga_d[:, :])
            vrow_sb = ctile([1, vcols], bf16, "vrow_sb")
            nc.sync.dma_start(out=vrow_sb, in_=vrow_d[:, :])
            imgb_sb = ctile([128, IMG_B_COLS], bf16, "imgb_sb")
            nc.sync.dma_start(out=imgb_sb, in_=imgb_d[:, :])
            xres_sb = ctile([128, 4, HID], f32, "xres_sb")
            for t_ in range(4):
                nc.gpsimd.dma_start(out=xres_sb[:, t_, :], in_=xres_d[ts(t_, 128), :])

            def iseg(name):
                o = IMG_OFF[name]
                sb = imga_sb if name in ("wq", "wk", "wv") else imgb_sb
                if name in ("wq", "wk"):
                    return sb[:, o : o + 2048].rearrange("p (k c) -> p k c", k=KT)
                if name in ("wv", "wv2"):
                    return sb[:, o : o + 512].rearrange("p (k c) -> p k c", k=KT)
                if name == "wse":
                    return sb[:, o : o + 8192].rearrange("p (k c) -> p k c", k=KT)
                if name == "sf":
                    return sb[:, o : o + 8]
                return sb[:, o : o + 1024]

            def vseg(name):
                o, c = voff[name]
                return vrow_sb[0:1, o : o + c]

            sf_sb = iseg("sf")
            wq_sb = iseg("wq")
            wk_sb = iseg("wk")
            wv_sb = iseg("wv")
            wse_sb = iseg("wse")
            wv2_sb = iseg("wv2")
            wo2_sb = iseg("wo2")[0:D_HEAD, :]
            sq_sb = vseg("sq")
            sk_sb = vseg("sk")
            sv_sb = vseg("sv")
            cvec_sb = {}
            for nm, has in (("cq", has_cq), ("ck", has_ck), ("cv", has_cv)):
                if has:
                    cvec_sb[nm] = vseg(nm)
            bse_sb = vseg("bse") if has_bse else None
            bv2_sb = vseg("bv2") if has_bv2 else None
            w_comb = ctile([D_HEAD + 1, HID], bf16, "w_comb")
            nc.sync.dma_start(
                out=w_comb[0:D_HEAD, :], in_=imgb_sb[0:D_HEAD, IMG_OFF["wo"] : IMG_OFF["wo"] + 1024]
            )

            ident = ctile([128, 128], bf16, "ident")
            make_identity(nc, ident)
            utri = ctile([128, 128], bf16, "utri")
            make_upper_triangular(nc, utri, val=1.0, diag=True)
            onesK = ctile([128, 1], bf16, "onesK")
            nc.vector.memset(onesK, 1.0 / HID)
            ones1 = ctile([1, 1], bf16, "ones1")
            nc.vector.memset(ones1, 1.0)
            eps_sb = ctile([1, 1], f32, "eps_sb")
            nc.vector.memset(eps_sb, LN_EPS)

            vrows = ctile([128, T], f32, "vrows")  # p0=rstd p32=m2 p64=var p96=lnv
            m_bf = ctile([1, T], bf16, "m_bf")
            invr_bf = ctile([1, T], bf16, "invr_bf")
            RSTD = ctile([128, T], f32, "RSTD")
            qT = [ctile([128, T], bf16, f"qT{i}") for i in range(2)]
            kT = [ctile([128, T], bf16, f"kT{i}") for i in range(2)]
            vT = ctile([D_HEAD, T], bf16, "vT")
            v_aug = ctile([128, NQT, D_HEAD + 1], bf16, "v_aug")
            nc.vector.memset(v_aug, 1.0)
            MT_acc = ctile([D_HEAD, T], f32, "MT_acc")
            MT_aug = ctile([D_HEAD + 1, 4, 128], bf16, "MT_aug")
            nc.vector.memset(MT_aug[D_HEAD : D_HEAD + 1, :, :], 1.0)
            ctxT_sb = ctile([128, KT], bf16, "ctxT_sb")

            # ---- LN stats: m, E[x^2] via ones-matmuls over xT ----
            for half in range(2):
                hc0 = half * 1024
                m_ps = patile([1, 1024], f"m_ps{half}")
                ms_ps = patile([1, 1024], f"ms_ps{half}")
                for k in range(KT):
                    xsq = wtile([128, 1024], bf16, f"xsq{half}_{k}", "xsq", 2)
                    nc.vector.tensor_mul(
                        xsq, xt_sb[:, k, hc0 : hc0 + 1024], xt_sb[:, k, hc0 : hc0 + 1024]
                    )
                    for n in range(2):
                        c0 = n * 512
                        nc.tensor.matmul(
                            m_ps[0:1, c0 : c0 + 512],
                            onesK,
                            xt_sb[:, k, hc0 + c0 : hc0 + c0 + 512],
                            start=(k == 0),
                            stop=(k == KT - 1),
                        )
                        nc.tensor.matmul(
                            ms_ps[0:1, c0 : c0 + 512],
                            onesK,
                            xsq[:, c0 : c0 + 512],
                            start=(k == 0),
                            stop=(k == KT - 1),
                        )
                nc.scalar.activation(m_bf[0:1, hc0 : hc0 + 1024], m_ps, AF.Copy)
                nc.scalar.activation(vrows[32:33, hc0 : hc0 + 1024], m_ps, AF.Square)
                nc.vector.tensor_sub(
                    vrows[64:65, hc0 : hc0 + 1024], ms_ps, vrows[32:33, hc0 : hc0 + 1024]
                )
            nc.scalar.activation(vrows[96:97, :], vrows[64:65, :], AF.Ln, bias=eps_sb[0:1, 0:1])
            nc.scalar.activation(vrows[0:1, :], vrows[96:97, :], AF.Exp, scale=-0.5)
            nc.scalar.activation(invr_bf[0:1, :], vrows[96:97, :], AF.Exp, scale=0.5)
            nc.gpsimd.partition_broadcast(RSTD, vrows[0:1, :])

            # ---- projections qT/kT/vT (transposed, LN folded) ----
            projs = [
                ("q", wq_sb, sq_sb, cvec_sb.get("cq"), [qT[0], qT[1]], 128),
                ("k", wk_sb, sk_sb, cvec_sb.get("ck"), [kT[0], kT[1]], 128),
                ("v", wv_sb, sv_sb, cvec_sb.get("cv"), [vT], 64),
            ]
            for nm, w_sb, s_sb, c_sb, dests, P in projs:
                for mc, dest in enumerate(dests):
                    mcols = slice(mc * 128, mc * 128 + P)
                    for half in range(2):
                        hc0 = half * 1024
                        pp = patile([P, 1024], f"pp_{nm}{mc}{half}")
                        for n in range(2):
                            c0 = n * 512
                            for k in range(KT):
                                nc.tensor.matmul(
                                    pp[:, c0 : c0 + 512],
                                    w_sb[:, k, mcols],
                                    xt_sb[:, k, hc0 + c0 : hc0 + c0 + 512],
                                    start=(k == 0),
                                    stop=False,
                                )
                            nc.tensor.matmul(
                                pp[:, c0 : c0 + 512],
                                s_sb[:, mcols],
                                m_bf[0:1, hc0 + c0 : hc0 + c0 + 512],
                                start=False,
                                stop=(c_sb is None),
                            )
                            if c_sb is not None:
                                nc.tensor.matmul(
                                    pp[:, c0 : c0 + 512],
                                    c_sb[:, mcols],
                                    invr_bf[0:1, hc0 + c0 : hc0 + c0 + 512],
                                    start=False,
                                    stop=True,
                                )
                        nc.vector.tensor_mul(
                            dest[:P, hc0 : hc0 + 1024], pp, RSTD[:P, hc0 : hc0 + 1024]
                        )

            # ---- v_aug = [v | 1] in natural layout via PE transposes ----
            for t_ in range(NQT):
                vt_ps = ps_a.tile([128, D_HEAD], bf16, name=f"vt_ps{t_}", tag="a")
                nc.tensor.transpose(vt_ps, vT[0:D_HEAD, ts(t_, 128)], ident[0:D_HEAD, 0:D_HEAD])
                nc.vector.tensor_copy(v_aug[:, t_, 0:D_HEAD], vt_ps)

            # ---- static path: so2 = (relu(sf@Wse+bse) @ Wv2 + bv2) @ Wo2 ----
            ctx_ps = patile([1, HID], "ctx_ps")
            for n in range(2):
                c0 = n * 512
                for k in range(KT):
                    nc.tensor.matmul(
                        ctx_ps[0:1, c0 : c0 + 512],
                        sf_sb[:, k : k + 1],
                        wse_sb[:, k, c0 : c0 + 512],
                        start=(k == 0),
                        stop=(k == KT - 1 and not has_bse),
                    )
                if has_bse:
                    nc.tensor.matmul(
                        ctx_ps[0:1, c0 : c0 + 512],
                        ones1,
                        bse_sb[:, c0 : c0 + 512],
                        start=False,
                        stop=True,
                    )
            ctx_sb = ctile([1, HID], bf16, "ctx_sb")
            nc.scalar.activation(ctx_sb, ctx_ps, AF.Relu)
            ctx_bounce = dpool.tile([1, HID], bf16, name="ctx_bounce", tag="ctx_bounce")
            nc.gpsimd.dma_start(out=ctx_bounce[:, :], in_=ctx_sb[:, :])
            nc.gpsimd.dma_start(
                out=ctxT_sb[:, :], in_=ctx_bounce[0, :].rearrange("(k p) -> p k", p=128)
            )
            v2_ps = ps_a.tile([1, D_HEAD], f32, name="v2_ps", tag="a")
            for k in range(KT):
                nc.tensor.matmul(
                    v2_ps,
                    ctxT_sb[:, k : k + 1],
                    wv2_sb[:, k, :],
                    start=(k == 0),
                    stop=(k == KT - 1 and not has_bv2),
                )
            if has_bv2:
                nc.tensor.matmul(v2_ps, ones1, bv2_sb, start=False, stop=True)
            v2_sb = ctile([1, D_HEAD], bf16, "v2_sb")
            nc.scalar.activation(v2_sb, v2_ps, AF.Copy)
            v2T_ps = ps_a.tile([D_HEAD, 1], bf16, name="v2T_ps", tag="a")
            nc.tensor.transpose(v2T_ps, v2_sb[0:1, :], ident[0:1, 0:1])
            v2T_sb = ctile([D_HEAD, 1], bf16, "v2T_sb")
            nc.vector.tensor_copy(v2T_sb, v2T_ps)
            so2_ps = patile([1, HID], "so2_ps")
            for n in range(2):
                c0 = n * 512
                nc.tensor.matmul(
                    so2_ps[0:1, c0 : c0 + 512], v2T_sb, wo2_sb[:, c0 : c0 + 512],
                    start=True, stop=True,
                )
            nc.scalar.activation(w_comb[D_HEAD : D_HEAD + 1, :], so2_ps, AF.Copy)

            # ---- attention (phase B): 512-col q-chunks, head-pair row-packed ----
            _phase_a.close()
            _phase_b = ExitStack()
            ps_s = _phase_b.enter_context(tc.tile_pool(name="ps_s", bufs=2, space="PSUM"))
            ps_u = _phase_b.enter_context(tc.tile_pool(name="ps_u", bufs=4, space="PSUM"))

            def launch_rs(c):
                cbase = 512 * c
                cc_in = dpool.tile([4, D_HEAD, 128], f32, name=f"cc_in{c}", tag=f"cc_in{c}")
                for blk in range(4):
                    nc.gpsimd.dma_start(
                        out=cc_in[blk],
                        in_=MT_acc[:, cbase + 128 * blk : cbase + 128 * blk + 128],
                    )
                cc_out = dpool.tile([D_HEAD, 128], f32, name=f"cc_out{c}", tag=f"cc_out{c}")
                nc.gpsimd.collective_compute(
                    "ReduceScatter",
                    ALU.add,
                    replica_groups=RG,
                    ins=[cc_in.opt()],
                    outs=[cc_out.opt()],
                )
                mt_st = wtile([D_HEAD, 128], f32, f"mt_st{c}", "mt_st", 2)
                nc.sync.dma_start(out=mt_st, in_=cc_out[:, :])
                nc.vector.tensor_copy(MT_aug[0:D_HEAD, c, :], mt_st)

            def do_outproj(c):
                out_sb = wtile([128, HID], f32, f"out_sb{c}", "out_sb", 2)
                for n in range(2):
                    c0 = n * 512
                    o_ps = putile([128, 512], f32, f"o_ps{c}{n}")
                    nc.tensor.matmul(
                        o_ps,
                        MT_aug[:, c, :],
                        w_comb[:, c0 : c0 + 512],
                        start=True,
                        stop=True,
                    )
                    nc.vector.tensor_add(
                        out_sb[:, c0 : c0 + 512], o_ps, xres_sb[:, c, c0 : c0 + 512]
                    )
                nc.sync.dma_start(out=out_d[ts(c, 128), :], in_=out_sb)

            chunk_order = [3, 2, 1, 0]
            for ci, c in enumerate(chunk_order):
                cbase = 512 * c
                for hp in range(2):
                    U = [
                        [putile([D_HEAD + 1, 512], f32, f"U{c}{hp}{hb}{s}") for s in range(1)]
                        for hb in range(2)
                    ]
                    jmax = 4 * c + 4
                    for j in range(jmax):
                        qs = max(cbase, j * 128)
                        N = cbase + 512 - qs
                        Sp = pstile([128, 1024], f32, f"S{c}{hp}{j}")
                        for hb in range(2):
                            rb = hb * 64
                            nc.tensor.matmul(
                                Sp[:, hb * 512 : hb * 512 + N],
                                kT[hp][rb : rb + 64, ts(j, 128)],
                                qT[hp][rb : rb + 64, qs : qs + N],
                                start=True,
                                stop=True,
                            )
                        E = wtile([128, 1024], bf16, f"E{c}{hp}{j}", "E", 3)
                        if N == 512:
                            nc.scalar.activation(E, Sp, AF.Exp)
                        else:
                            nc.scalar.activation(E[:, 0:N], Sp[:, 0:N], AF.Exp)
                            nc.scalar.activation(
                                E[:, 512 : 512 + N], Sp[:, 512 : 512 + N], AF.Exp
                            )
                        if j >= 4 * c:
                            nc.vector.tensor_mul(E[:, 0:128], E[:, 0:128], utri)
                            nc.vector.tensor_mul(E[:, 512:640], E[:, 512:640], utri)
                        u0 = qs - cbase
                        j_last = jmax - 1
                        for hb in range(2):
                            nc.tensor.matmul(
                                U[hb][0][:, u0 : u0 + N],
                                v_aug[:, j, :],
                                E[:, hb * 512 : hb * 512 + N],
                                start=(j == 0),
                                stop=(j == j_last),
                                skip_group_check=True,
                            )
                    for hb in range(2):
                        first_head = hp == 0 and hb == 0
                        gs = slice(cbase, cbase + 512)
                        dnr = wtile([1, 512], f32, f"dnr{c}{hp}{hb}", "dnr", 2)
                        nc.vector.tensor_copy(dnr, U[hb][0][D_HEAD : D_HEAD + 1, :])
                        dn = wtile([1, 512], f32, f"dn{c}{hp}{hb}", "dn", 2)
                        nc.vector.reciprocal_approx_fast(dn, dnr)
                        bc = wtile([64, 512], f32, f"bc{c}{hp}{hb}", "bc", 2)
                        nc.gpsimd.partition_broadcast(bc, dn[0:1, :])
                        if first_head:
                            nc.vector.tensor_mul(MT_acc[:, gs], U[hb][0][0:D_HEAD, :], bc)
                        else:
                            tmp = wtile([64, 512], f32, f"tmp{c}{hp}{hb}", "tmp", 2)
                            nc.vector.tensor_mul(tmp, U[hb][0][0:D_HEAD, :], bc)
                            nc.vector.tensor_add(MT_acc[:, gs], MT_acc[:, gs], tmp)
                launch_rs(c)
                if ci >= 2:
                    do_outproj(chunk_order[ci - 2])
            do_outproj(chunk_order[2])
            do_outproj(chunk_order[3])
            _phase_b.close()

    nc.compile()
    return nc


def _get_nc(flags):
    if flags not in _nc_cache:
        _nc_cache[flags] = _build(flags)
    return _nc_cache[flags]


def _prep_core_inputs(c, arrs, flags):
    has_cq, has_ck, has_cv, has_bse, has_bv2 = flags
    b, p = divmod(c, GROUP)
    x = arrs["x"][b]  # [T, HID] f32
    g1 = arrs["g1"]
    hs = slice(p * HCOLS, (p + 1) * HCOLS)

    def kmajor(w):  # [HID, C] -> [128, KT*C] in k-major layout
        return np.ascontiguousarray(
            w.reshape(KT, 128, w.shape[1]).transpose(1, 0, 2).reshape(128, -1)
        )

    wq_full = (g1[:, None] * arrs["Wq1"]) / 8.0
    wk_full = g1[:, None] * arrs["Wk1"]
    wv_full = g1[:, None] * arrs["Wv1"]
    wq = wq_full[:, hs]
    wk = wk_full[:, hs]
    cq_full = (arrs["beta1"] @ arrs["Wq1"] + arrs["bq1"]) / 8.0
    ck_full = arrs["beta1"] @ arrs["Wk1"] + arrs["bk1"]
    cv_full = arrs["beta1"] @ arrs["Wv1"] + arrs["bv1"]

    imga = np.zeros((128, IMG_A_COLS), np.float32)
    imga[:, IMG_OFF["wq"] : IMG_OFF["wq"] + 2048] = kmajor(wq)
    imga[:, IMG_OFF["wk"] : IMG_OFF["wk"] + 2048] = kmajor(wk)
    imga[:, IMG_OFF["wv"] : IMG_OFF["wv"] + 512] = kmajor(wv_full)
    imgb = np.zeros((128, IMG_B_COLS), np.float32)
    imgb[:, IMG_OFF["sf"] : IMG_OFF["sf"] + KT] = arrs["static_features"][b].reshape(KT, 128).T
    imgb[:, IMG_OFF["wse"] : IMG_OFF["wse"] + 8192] = kmajor(arrs["Wse"])
    imgb[:, IMG_OFF["wv2"] : IMG_OFF["wv2"] + 512] = kmajor(arrs["Wv2"])
    imgb[0:D_HEAD, IMG_OFF["wo2"] : IMG_OFF["wo2"] + 1024] = arrs["Wo2"]
    imgb[0:D_HEAD, IMG_OFF["wo"] : IMG_OFF["wo"] + 1024] = arrs["Wo1"] / float(N_HEADS)

    voff, vcols = _vrow_layout(flags)
    vrow = np.zeros((1, vcols), np.float32)

    def putv(name, vec):
        o, n = voff[name]
        vrow[0, o : o + len(vec)] = vec

    putv("sq", -wq.astype(np.float64).sum(0))
    putv("sk", -wk.astype(np.float64).sum(0))
    putv("sv", -wv_full.astype(np.float64).sum(0))
    if has_cq:
        putv("cq", cq_full[hs])
    if has_ck:
        putv("ck", ck_full[hs])
    if has_cv:
        putv("cv", cv_full)
    if has_bse:
        putv("bse", arrs["bse"])
    if has_bv2:
        putv("bv2", arrs["bv2"])

    rows = np.r_[tuple(slice(512 * c + 128 * p, 512 * c + 128 * p + 128) for c in range(4))]
    return {
        "xt": np.ascontiguousarray(
            x.T.reshape(KT, 128, T).transpose(1, 0, 2)
        ).astype(BF),
        "xres": np.ascontiguousarray(x[rows]).astype(np.float32),
        "imga": imga.astype(BF),
        "imgb": imgb.astype(BF),
        "vrow": vrow.astype(BF),
    }


def kernel(**inputs):
    global last_results
    arrs = {k: np.asarray(v, np.float32) for k, v in inputs.items()}

    cq_full = (arrs["beta1"] @ arrs["Wq1"] + arrs["bq1"]) / 8.0
    ck_full = arrs["beta1"] @ arrs["Wk1"] + arrs["bk1"]
    cv_full = arrs["beta1"] @ arrs["Wv1"] + arrs["bv1"]
    flags = (
        bool(np.any(cq_full != 0)),
        bool(np.any(ck_full != 0)),
        bool(np.any(cv_full != 0)),
        bool(np.any(arrs["bse"] != 0)),
        bool(np.any(arrs["bv2"] != 0)),
    )
    nc = _get_nc(flags)

    in_maps = [_prep_core_inputs(c, arrs, flags) for c in range(NCORES)]

    from concourse.bass_utils import run_bass_kernel_spmd

    kw = {}
    prof_dir = os.environ.get("BASS_PROF_DIR")
    if prof_dir:
        os.makedirs(prof_dir, exist_ok=True)
        kw["tmpdir"] = prof_dir
    res = run_bass_kernel_spmd(nc, in_maps, list(range(NCORES)), **kw)
    last_results = res

    out = np.empty((BS, T, HID), np.float32)
    for core in range(NCORES):
        b, p = divmod(core, GROUP)
        o = np.asarray(res.results[core]["out"], np.float32)
        for c in range(4):
            out[b, 512 * c + 128 * p : 512 * c + 128 * p + 128] = o[128 * c : 128 * c + 128]
    return out
